# revision 3
# baseline (speedup 1.0000x reference)
"""GPT-J joint attention (B=1, S=2048, D=2048, H=16, HD=128) on 8 Trainium2
NeuronCores, tensor-parallel over heads (2 heads per core).

Per-core program (all matmuls bf16 inputs, fp32 PSUM accumulation):
  - QT/KT = W[qk]_shard @ hidden^T        ([hd, s] layout, per head)
  - RoPE applied via a rotation-matrix matmul + elementwise combine
  - V = hidden @ Wv_shard^T               ([s, hd] layout)
  - scores^T tiles = KT_tile^T . QT_block ([k, q] layout) -> exp -> causal mask
  - O^T accumulated as V_tile^T . P^T; softmax denominator via ones-matmul
  - partial out = O^T{normalized}^T . Wo_shard^T, streamed to DRAM per row-block

v2 structure: all DRAM inputs are pre-laid-out on the host so every DMA is a
contiguous copy; a dummy-matmul warmup chain un-throttles the PE clock (HAM)
while the first weights stream in; attention for block qb is interleaved right
after its projections; each block's softmax-denominator chain is emitted after
the NEXT block's projection matmuls so the PE never head-of-line blocks on DVE.

Host side: shard/transpose/cast inputs, run SPMD on 8 cores, sum the 8
partial outputs (the tensor-parallel all-reduce equivalent).
"""
import sys

import numpy as np
import ml_dtypes

try:
    import concourse.bass as bass
except ImportError:  # pragma: no cover
    sys.path.insert(0, "/opt/trn_rl_repo")
    import concourse.bass as bass

import concourse.mybir as mybir
import concourse.tile as tile
from concourse.bass_utils import run_bass_kernel_spmd

BF16 = mybir.dt.bfloat16
F32 = mybir.dt.float32
NPBF16 = ml_dtypes.bfloat16

N_CORES = 8
S = 2048          # sequence length
D = 2048          # model dim
HD = 128          # head dim
NHC = 2           # heads per core
DC = NHC * HD     # shard width (256)
P = 128           # partitions
KD = D // P       # 16 contraction tiles over model dim
QBS = 512         # q-block size
NQB = S // QBS    # 4 q-blocks
NST = S // P      # 16 sequence tiles of 128
SCALE = 1.0 / float(np.sqrt(HD))
N_WARM = 72       # dummy matmuls covering the pre-DMA window (~55ns each cold)

# ---------------------------------------------------------------------------
# Walrus's CoreV3 drain encoding accepts a single sem wait; Tile's tail drain
# carries one wait per logical proc. Split it into one drain per proc.
# ---------------------------------------------------------------------------


def _install_drain_split():
    if getattr(tile.TileContext, "_drain_split_installed", False):
        return
    from concourse.vector_clock import ScopedClock, VectorClock

    def _drain_and_barrier(self, tick_clock, wait_clock):
        full = tick_clock.global_clock
        n = len(full)
        for i in range(n):
            if full[i] <= 0:
                continue
            vec = [full[j] if j == i else 0 for j in range(n)]
            drain_inst = self.nc.sync.drain()
            wait_clock.add_sem_waits(
                drain_inst.ins, ScopedClock({None: VectorClock(vec)})
            )
        self.nc.all_engine_barrier()
        assert self.sems is not None
        popped = self.nc._tile_sem_poison_stack.pop()
        assert popped is self._sem_poison
        self.nc.clear_and_free_semaphores(list(self.sems.allocated().values()))
        self.nc.all_engine_barrier()

    tile.TileContext._drain_and_barrier = _drain_and_barrier
    tile.TileContext._drain_split_installed = True


def _split_excess_waits(nc, limit=1):
    """This walrus build rejects instructions carrying more than one sem wait
    (CoreV3 setupSyncWait: 'Too many sync wait commands'). Spill excess waits
    onto same-engine NOPs inserted just before the instruction — the engine
    executes them in queue order, so blocking semantics are unchanged."""
    ctr = 0
    for fn in nc.m.functions:
        for blk in fn.blocks:
            new_list = []
            for inst in blk.instructions:
                si = inst.sync_info
                if si is not None and len(si.on_wait) > limit:
                    waits = list(si.on_wait)
                    excess, keep = waits[:-limit], waits[-limit:]
                    for w in excess:
                        ctr += 1
                        nop = mybir.InstNoOp(
                            name=f"I-wsplit-{ctr}", text_hint="wait_split"
                        )
                        nop.engine = inst.engine
                        nop.sync_info = mybir.SyncInfo(on_wait=[w], on_update=[])
                        new_list.append(nop)
                    inst.sync_info = mybir.SyncInfo(
                        on_wait=keep, on_update=si.on_update
                    )
                new_list.append(inst)
            if len(new_list) != len(blk.instructions):
                blk.instructions[:] = new_list
    return ctr


def build_nc(split_waits=True):
    _install_drain_split()
    nc = bass.Bass()

    # All inputs are host-pre-laid-out so each DMA below is a contiguous copy.
    hT = nc.dram_tensor("hT", [P, NQB, KD, QBS], BF16, kind="ExternalInput")
    wq = nc.dram_tensor("wq", [P, KD, DC], BF16, kind="ExternalInput")
    wk = nc.dram_tensor("wk", [P, KD, DC], BF16, kind="ExternalInput")
    wv = nc.dram_tensor("wv", [P, KD, DC], BF16, kind="ExternalInput")
    wo = nc.dram_tensor("wo", [P, NHC, D], BF16, kind="ExternalInput")
    ct = nc.dram_tensor("ct", [P, S], BF16, kind="ExternalInput")
    st = nc.dram_tensor("st", [P, S], BF16, kind="ExternalInput")
    rot = nc.dram_tensor("rot", [P, P], BF16, kind="ExternalInput")
    out = nc.dram_tensor("out", [S, D], BF16, kind="ExternalOutput")

    Exp = mybir.ActivationFunctionType.Exp
    Copy = mybir.ActivationFunctionType.Copy

    with tile.TileContext(nc) as tc:
        with (
            tc.tile_pool(name="const", bufs=1) as const,
            tc.tile_pool(name="acts", bufs=1) as acts,
            tc.tile_pool(name="work", bufs=2) as work,
            tc.tile_pool(name="ptpool", bufs=5) as ptpool,
            tc.tile_pool(name="outstage", bufs=3) as outstage,
            tc.tile_pool(name="ps_score", bufs=4, space="PSUM") as ps_score,
            tc.tile_pool(name="ps_op", bufs=2, space="PSUM") as ps_op,
            tc.tile_pool(name="ps_acc", bufs=2, space="PSUM") as ps_acc,
        ):
            # ---- PE warmup: HAM un-throttles after ~3.4us of sustained PE
            # activity; run a dummy matmul chain while the first DMAs land so
            # the real projection stream starts at full clock. ----
            wdum = const.tile([P, 64], BF16)
            nc.vector.memset(wdum, 0.25)
            wps = ps_acc.tile([P, QBS], F32, name="warm", tag="ps_ot")
            for _ in range(N_WARM):
                nc.tensor.matmul(
                    wps[0:64, 0:64], lhsT=wdum, rhs=wdum[:, 0:64],
                    start=True, stop=True, skip_group_check=True,
                )

            # ---- constants / weights into SBUF; order = first-use order ----
            wq_sb = const.tile([P, KD, DC], BF16)
            wk_sb = const.tile([P, KD, DC], BF16)
            wv_sb = const.tile([P, KD, DC], BF16)
            hT_sb = const.tile([P, NQB, KD, QBS], BF16)
            ct_sb = const.tile([P, S], BF16)
            st_sb = const.tile([P, S], BF16)
            rot_sb = const.tile([P, P], BF16)
            wo_sb = const.tile([P, NHC, D], BF16)
            nc.sync.dma_start(out=wq_sb[:, 0:8, :], in_=wq[:, 0:8, :])
            nc.sync.dma_start(out=wq_sb[:, 8:16, :], in_=wq[:, 8:16, :])
            nc.sync.dma_start(out=rot_sb, in_=rot[:, :])
            nc.sync.dma_start(out=ct_sb[:, 0:QBS], in_=ct[:, 0:QBS])
            nc.sync.dma_start(out=st_sb[:, 0:QBS], in_=st[:, 0:QBS])
            for c4 in range(4):
                nc.sync.dma_start(
                    out=hT_sb[:, 0, c4 * 4:(c4 + 1) * 4, :],
                    in_=hT[:, 0, c4 * 4:(c4 + 1) * 4, :],
                )
            nc.sync.dma_start(out=wk_sb, in_=wk[:, :, :])
            nc.sync.dma_start(out=ct_sb[:, QBS:], in_=ct[:, QBS:])
            nc.sync.dma_start(out=st_sb[:, QBS:], in_=st[:, QBS:])
            nc.sync.dma_start(out=wv_sb, in_=wv[:, :, :])
            for qb in range(1, NQB):
                nc.sync.dma_start(out=hT_sb[:, qb], in_=hT[:, qb])
            nc.sync.dma_start(out=wo_sb, in_=wo[:, :, :])
            ones_colb = const.tile([P, 1], BF16)   # lhsT for k-partition sums
            nc.vector.memset(ones_colb, 1.0)
            ones_row = const.tile([1, P], BF16)    # lhsT for partition broadcast
            nc.vector.memset(ones_row, 1.0)

            # persistent activations
            qt_sb = acts.tile([P, NHC, S], BF16)   # [hd, h, s] rotary-applied Q^T
            kt_sb = acts.tile([P, NHC, S], BF16)
            v_sb = acts.tile([P, NST, DC], BF16)   # [s%128, s//128, head*hd]
            otb_sb = acts.tile([P, NHC, S], BF16)  # normalized O^T per head

            # ---- projections; the rope epilogue (psum->sbuf copy, rotation
            # matmul, 3 DVE ops) of each 16-matmul block is deferred until
            # after the NEXT block's matmuls are emitted, so PE never waits. ----
            pending = []  # (psum, dst_sb, h, qb)

            def flush_rope(keep=0):
                while len(pending) > keep:
                    ps, dst_sb, h, qb = pending.pop(0)
                    sl = slice(qb * QBS, (qb + 1) * QBS)
                    raw = work.tile([P, QBS], BF16, tag="raw")
                    nc.scalar.activation(raw, ps, Copy)
                    rps = ps_score.tile([P, QBS], F32, tag="mm")
                    nc.tensor.matmul(rps, lhsT=rot_sb, rhs=raw, start=True, stop=True)
                    t1 = work.tile([P, QBS], BF16, tag="t1")
                    t2 = work.tile([P, QBS], BF16, tag="t2")
                    nc.vector.tensor_mul(t1, raw, ct_sb[:, sl])
                    nc.vector.tensor_mul(t2, rps, st_sb[:, sl])
                    nc.vector.tensor_add(dst_sb[:, h, sl], t1, t2)

            def project(w_sb, dst_sb, h, qb):
                flush_rope(keep=1)
                ps = ps_score.tile([P, QBS], F32, name="proj_ps", tag="mm")
                for kd in range(KD):
                    nc.tensor.matmul(
                        ps,
                        lhsT=w_sb[:, kd, h * HD:(h + 1) * HD],
                        rhs=hT_sb[:, qb, kd, :],
                        start=(kd == 0),
                        stop=(kd == KD - 1),
                    )
                pending.append((ps, dst_sb, h, qb))

            def proj_block(qb):
                for h in range(NHC):
                    project(wq_sb, qt_sb, h, qb)
                for h in range(NHC):
                    project(wk_sb, kt_sb, h, qb)
                for s4 in range(4):
                    flush_rope(keep=1)
                    st_idx = qb * 4 + s4
                    ps = ps_op.tile([P, QBS], F32, tag="op")
                    for kd in range(KD):
                        nc.tensor.matmul(
                            ps[:, 0:DC],
                            lhsT=hT_sb[:, qb, kd, s4 * P:(s4 + 1) * P],
                            rhs=wv_sb[:, kd, :],
                            start=(kd == 0),
                            stop=(kd == KD - 1),
                        )
                    nc.scalar.activation(v_sb[:, st_idx, :], ps[:, 0:DC], Copy)
                flush_rope()

            # ---- attention; kt loop per q-block; denominator chain is
            # emitted later (after the next block's projections) ----
            def out_proj(qb, s4s=range(4), tail=False):
                for s4 in s4s:
                    st_idx = qb * 4 + s4
                    ost = outstage.tile([P, D], BF16, tag="ost")
                    for eb in range(NQB):
                        ops = ps_op.tile([P, QBS], F32, name="ops", tag="op")
                        for h in range(NHC):
                            nc.tensor.matmul(
                                ops,
                                lhsT=otb_sb[:, h, st_idx * P:(st_idx + 1) * P],
                                rhs=wo_sb[:, h, eb * QBS:(eb + 1) * QBS],
                                start=(h == 0),
                                stop=(h == NHC - 1),
                            )
                        osl = ost[:, eb * QBS:(eb + 1) * QBS]
                        if tail and eb % 2 == 0:
                            nc.scalar.activation(osl, ops, Copy)
                        else:
                            nc.vector.tensor_copy(osl, ops)
                        if tail and eb == 1:
                            nc.sync.dma_start(
                                out=out[st_idx * P:(st_idx + 1) * P, 0:2 * QBS],
                                in_=ost[:, 0:2 * QBS],
                            )
                    if tail:
                        nc.sync.dma_start(
                            out=out[st_idx * P:(st_idx + 1) * P, 2 * QBS:],
                            in_=ost[:, 2 * QBS:],
                        )
                    else:
                        nc.sync.dma_start(
                            out=out[st_idx * P:(st_idx + 1) * P, :], in_=ost
                        )

            OT_LAG = 3  # P.V matmul trails the score matmul so its sem wait
            # is already satisfied and LDWEIGHTS pipelines.
            blk = {}  # per-qb state carried from kt loop to den chain

            def att_ktloop(qb):
                qsl = slice(qb * QBS, (qb + 1) * QBS)
                kmax = (qb + 1) * 4
                ot_pss, pts = [], {}
                # 4-lane bf16 partial sums of exp tiles (softmax denominator);
                # all adds run in the DVE 16-bit fast mode, chains stay short.
                accs = [[None] * 4 for _ in range(NHC)]

                def acc_pt(h, kt, pt):
                    lane = kt % 4
                    if accs[h][lane] is None:
                        acc = work.tile(
                            [P, QBS], BF16, name=f"za{h}_{lane}",
                            tag=f"za{h}_{lane}",
                        )
                        nc.vector.tensor_copy(acc, pt)
                        accs[h][lane] = acc
                    else:
                        acc = accs[h][lane]
                        nc.vector.tensor_add(acc, acc, pt)

                for h in range(NHC):
                    ot_pss.append(ps_acc.tile([P, QBS], F32, name="ot_ps", tag="ps_ot"))

                kt_order = list(range(qb * 4, kmax)) + list(range(0, qb * 4))

                def pv_step(kt):
                    for h in range(NHC):
                        nc.tensor.matmul(
                            ot_pss[h],
                            lhsT=v_sb[:, kt, h * HD:(h + 1) * HD],
                            rhs=pts[(h, kt)],
                            start=(kt == kt_order[0]),
                            stop=(kt == kt_order[-1]),
                            skip_group_check=True,
                        )

                for ki, kt in enumerate(kt_order):
                    for h in range(NHC):
                        sps = ps_score.tile([P, QBS], F32, tag="mm")
                        nc.tensor.matmul(
                            sps,
                            lhsT=kt_sb[:, h, kt * P:(kt + 1) * P],
                            rhs=qt_sb[:, h, qsl],
                            start=True,
                            stop=True,
                        )
                        pt = ptpool.tile([P, QBS], BF16, tag=f"pt{h}")
                        nc.scalar.activation(pt, sps, Exp, scale=SCALE)
                        j = kt - qb * 4
                        if j >= 0:  # diagonal tile: causal mask (on Pool)
                            w = min(P * (j + 1), QBS)
                            nc.gpsimd.affine_select(
                                out=pt[:, 0:w],
                                in_=pt[:, 0:w],
                                compare_op=mybir.AluOpType.is_ge,
                                fill=0.0,
                                base=qb * QBS - kt * P,
                                pattern=[[1, w]],
                                channel_multiplier=-1,
                            )
                        pts[(h, kt)] = pt
                        acc_pt(h, kt, pt)
                    if ki >= OT_LAG:
                        pv_step(kt_order[ki - OT_LAG])
                for ki2 in range(max(kmax - OT_LAG, 0), kmax):
                    pv_step(kt_order[ki2])
                blk[qb] = (ot_pss, accs)

            def att_den(qb):
                qsl = slice(qb * QBS, (qb + 1) * QBS)
                ot_pss, accs = blk.pop(qb)
                den_sbs = []
                for h in range(NHC):
                    # merge the 4 bf16 lanes on DVE, then one ones-matmul
                    lanes = [a for a in accs[h] if a is not None]
                    while len(lanes) > 1:
                        nxt = []
                        for i in range(0, len(lanes) - 1, 2):
                            nc.vector.tensor_add(lanes[i], lanes[i], lanes[i + 1])
                            nxt.append(lanes[i])
                        if len(lanes) % 2:
                            nxt.append(lanes[-1])
                        lanes = nxt
                    den_ps = ps_op.tile([P, QBS], F32, name="den_ps", tag="op")
                    nc.tensor.matmul(
                        den_ps[0:1, :], lhsT=ones_colb, rhs=lanes[0],
                        start=True, stop=True, skip_group_check=True,
                    )
                    r_sb = work.tile([1, QBS], F32, tag=f"r{h}")
                    nc.vector.reciprocal(r_sb, den_ps[0:1, :])
                    r_bf = work.tile([1, QBS], BF16, tag=f"rb{h}")
                    nc.vector.tensor_copy(r_bf, r_sb)
                    den_sbs.append(r_bf)
                for h in range(NHC):
                    bc_ps = ps_op.tile([P, QBS], F32, name="bc_ps", tag="op")
                    nc.tensor.matmul(
                        bc_ps, lhsT=ones_row, rhs=den_sbs[h], start=True, stop=True
                    )
                    bc_sb = work.tile([P, QBS], F32, tag=f"bc{h}")
                    nc.vector.tensor_copy(bc_sb, bc_ps)
                    nc.vector.tensor_mul(otb_sb[:, h, qsl], ot_pss[h], bc_sb)

            # ---- main schedule ----
            proj_block(0)
            att_ktloop(0)
            for qb in range(1, NQB):
                proj_block(qb)
                att_den(qb - 1)
                if qb >= 2:
                    out_proj(qb - 2)
                att_ktloop(qb)
            out_proj(NQB - 2)  # fills the PE while den(3)'s DVE chain runs
            att_den(NQB - 1)
            out_proj(NQB - 1, tail=True)
    if split_waits:
        _split_excess_waits(nc)
    return nc


_NC_CACHE = {}


def _get_nc():
    if "nc" not in _NC_CACHE:
        _NC_CACHE["nc"] = build_nc()
    return _NC_CACHE["nc"]


def _rotation_matrix_T():
    # rot(x)[2i] = -x[2i+1]; rot(x)[2i+1] = x[2i].  R[i,j] coefficient of x[j].
    R = np.zeros((HD, HD), np.float32)
    idx = np.arange(0, HD, 2)
    R[idx, idx + 1] = -1.0
    R[idx + 1, idx] = 1.0
    return np.ascontiguousarray(R.T)


def prepare_in_maps(hidden_states, sin, cos, Wq, Wk, Wv, Wo):
    hidden_states = np.asarray(hidden_states, dtype=np.float32)
    sin = np.asarray(sin, dtype=np.float32)
    cos = np.asarray(cos, dtype=np.float32)
    Wq = np.asarray(Wq, dtype=np.float32)
    Wk = np.asarray(Wk, dtype=np.float32)
    Wv = np.asarray(Wv, dtype=np.float32)
    Wo = np.asarray(Wo, dtype=np.float32)

    hT = hidden_states[0].T.astype(NPBF16)  # [D, S]
    hT4 = np.ascontiguousarray(
        hT.reshape(KD, P, NQB, QBS).transpose(1, 2, 0, 3)
    )  # [P, NQB, KD, QBS]
    ct = np.ascontiguousarray(np.repeat(cos, 2, axis=1).T).astype(NPBF16)
    st = np.ascontiguousarray(np.repeat(sin, 2, axis=1).T).astype(NPBF16)
    rot = _rotation_matrix_T().astype(NPBF16)

    in_maps = []
    for c in range(N_CORES):
        e0 = c * DC
        wq_p = np.ascontiguousarray(
            Wq[e0:e0 + DC, :].T.astype(NPBF16).reshape(KD, P, DC).transpose(1, 0, 2)
        )
        wk_p = np.ascontiguousarray(
            Wk[e0:e0 + DC, :].T.astype(NPBF16).reshape(KD, P, DC).transpose(1, 0, 2)
        )
        wv_p = np.ascontiguousarray(
            Wv[e0:e0 + DC, :].T.astype(NPBF16).reshape(KD, P, DC).transpose(1, 0, 2)
        )
        wo_p = np.ascontiguousarray(
            Wo[:, e0:e0 + DC].T.astype(NPBF16).reshape(NHC, P, D).transpose(1, 0, 2)
        )
        in_maps.append(
            {
                "hT": hT4,
                "wq": wq_p,
                "wk": wk_p,
                "wv": wv_p,
                "wo": wo_p,
                "ct": ct,
                "st": st,
                "rot": rot,
            }
        )
    return in_maps


def kernel(hidden_states, attention_mask, sin, cos, Wq, Wk, Wv, Wo):
    in_maps = prepare_in_maps(hidden_states, sin, cos, Wq, Wk, Wv, Wo)
    nc = _get_nc()
    res = run_bass_kernel_spmd(nc, in_maps, list(range(N_CORES)))
    out = res.results[0]["out"].astype(np.float32)
    for c in range(1, N_CORES):
        out += res.results[c]["out"].astype(np.float32)
    return out[None]


# revision 6
# speedup vs baseline: 1.1364x; 1.1364x over previous
"""GPT-J joint attention (B=1, S=2048, D=2048, H=16, HD=128) on 8 Trainium2
NeuronCores, tensor-parallel over heads (2 heads per core).

Per-core program (all matmuls bf16 inputs, fp32 PSUM accumulation):
  - QT/KT = W[qk]_shard @ hidden^T        ([hd, s] layout, per head)
  - RoPE applied via a rotation-matrix matmul + elementwise combine
  - V = hidden @ Wv_shard^T               ([s, hd] layout)
  - scores^T tiles = KT_tile^T . QT_block ([k, q] layout) -> exp -> causal
    mask via a precomputed 0/1 mask multiply on DVE
  - O^T accumulated as V_tile^T . P^T; softmax denominator via ones-matmul
  - partial out = O^T{normalized}^T . Wo_shard^T, streamed to DRAM per row-block

v3 structure: host pre-lays-out all inputs so every DMA is contiguous; a
dummy-matmul warmup chain un-throttles the PE clock (HAM) while the first
weights stream in; Q/K projection chains for both heads interleave per-kd so
chunked hT DMA keeps up; attention for block qb runs right after its
projections; each block's softmax-denominator chain is emitted after later
independent PE work so the PE never head-of-line blocks on DVE; out-proj
row-groups are spread through the next attention loops to smooth the
psum->sbuf copy load.

Host side: shard/transpose/cast inputs, run SPMD on 8 cores, sum the 8
partial outputs (the tensor-parallel all-reduce equivalent).
"""
import sys

import numpy as np
import ml_dtypes

try:
    import concourse.bass as bass
except ImportError:  # pragma: no cover
    sys.path.insert(0, "/opt/trn_rl_repo")
    import concourse.bass as bass

import concourse.mybir as mybir
import concourse.tile as tile
from concourse.bass_utils import run_bass_kernel_spmd

BF16 = mybir.dt.bfloat16
F32 = mybir.dt.float32
NPBF16 = ml_dtypes.bfloat16

N_CORES = 8
S = 2048          # sequence length
D = 2048          # model dim
HD = 128          # head dim
NHC = 2           # heads per core
DC = NHC * HD     # shard width (256)
P = 128           # partitions
KD = D // P       # 16 contraction tiles over model dim
QBS = 512         # q-block size
NQB = S // QBS    # 4 q-blocks
NST = S // P      # 16 sequence tiles of 128
SCALE = 1.0 / float(np.sqrt(HD))
N_WARM = 60       # dummy matmuls covering the pre-DMA window (~55ns each cold)

# ---------------------------------------------------------------------------
# Walrus's CoreV3 drain encoding accepts a single sem wait; Tile's tail drain
# carries one wait per logical proc. Split it into one drain per proc.
# ---------------------------------------------------------------------------


def _install_drain_split():
    if getattr(tile.TileContext, "_drain_split_installed", False):
        return
    from concourse.vector_clock import ScopedClock, VectorClock

    def _drain_and_barrier(self, tick_clock, wait_clock):
        full = tick_clock.global_clock
        n = len(full)
        for i in range(n):
            if full[i] <= 0:
                continue
            vec = [full[j] if j == i else 0 for j in range(n)]
            drain_inst = self.nc.sync.drain()
            wait_clock.add_sem_waits(
                drain_inst.ins, ScopedClock({None: VectorClock(vec)})
            )
        self.nc.all_engine_barrier()
        assert self.sems is not None
        popped = self.nc._tile_sem_poison_stack.pop()
        assert popped is self._sem_poison
        self.nc.clear_and_free_semaphores(list(self.sems.allocated().values()))
        self.nc.all_engine_barrier()

    tile.TileContext._drain_and_barrier = _drain_and_barrier
    tile.TileContext._drain_split_installed = True


def _split_excess_waits(nc, limit=1):
    """This walrus build rejects instructions carrying more than one sem wait
    (CoreV3 setupSyncWait: 'Too many sync wait commands'). Spill excess waits
    onto same-engine NOPs inserted just before the instruction — the engine
    executes them in queue order, so blocking semantics are unchanged."""
    ctr = 0
    for fn in nc.m.functions:
        for blk in fn.blocks:
            new_list = []
            for inst in blk.instructions:
                si = inst.sync_info
                if si is not None and len(si.on_wait) > limit:
                    waits = list(si.on_wait)
                    excess, keep = waits[:-limit], waits[-limit:]
                    for w in excess:
                        ctr += 1
                        nop = mybir.InstNoOp(
                            name=f"I-wsplit-{ctr}", text_hint="wait_split"
                        )
                        nop.engine = inst.engine
                        nop.sync_info = mybir.SyncInfo(on_wait=[w], on_update=[])
                        new_list.append(nop)
                    inst.sync_info = mybir.SyncInfo(
                        on_wait=keep, on_update=si.on_update
                    )
                new_list.append(inst)
            if len(new_list) != len(blk.instructions):
                blk.instructions[:] = new_list
    return ctr


def build_nc(split_waits=True):
    _install_drain_split()
    nc = bass.Bass()

    # All inputs are host-pre-laid-out so each DMA below is a contiguous copy.
    hT = nc.dram_tensor("hT", [P, NQB, KD, QBS], BF16, kind="ExternalInput")
    wq = nc.dram_tensor("wq", [P, KD, DC], BF16, kind="ExternalInput")
    wk = nc.dram_tensor("wk", [P, KD, DC], BF16, kind="ExternalInput")
    wv = nc.dram_tensor("wv", [P, KD, DC], BF16, kind="ExternalInput")
    wo = nc.dram_tensor("wo", [P, NHC, D], BF16, kind="ExternalInput")
    # rope[p, qb, 0, :] = cos row p of q-block qb; rope[p, qb, 1, :] = sin
    rope = nc.dram_tensor("rope", [P, NQB, 2, QBS], BF16, kind="ExternalInput")
    rot = nc.dram_tensor("rot", [P, P], BF16, kind="ExternalInput")
    # cmask[k, j, x] = 1.0 if x >= j*128 + k else 0 (causal mask, diag tile j)
    cmask = nc.dram_tensor("cmask", [P, 4, QBS], BF16, kind="ExternalInput")
    out = nc.dram_tensor("out", [S, D], BF16, kind="ExternalOutput")

    Exp = mybir.ActivationFunctionType.Exp
    Copy = mybir.ActivationFunctionType.Copy
    Ln = mybir.ActivationFunctionType.Ln

    with tile.TileContext(nc) as tc:
        with (
            tc.tile_pool(name="const", bufs=1) as const,
            tc.tile_pool(name="acts", bufs=1) as acts,
            tc.tile_pool(name="work", bufs=2) as work,
            tc.tile_pool(name="ptpool", bufs=6) as ptpool,
            tc.tile_pool(name="outstage", bufs=3) as outstage,
            tc.tile_pool(name="ps_score", bufs=4, space="PSUM") as ps_score,
            tc.tile_pool(name="ps_op", bufs=2, space="PSUM") as ps_op,
            tc.tile_pool(name="ps_acc", bufs=2, space="PSUM") as ps_acc,
        ):
            # ---- PE warmup: HAM un-throttles after ~3.4us of sustained PE
            # activity; run a dummy matmul chain while the first DMAs land so
            # the real projection stream starts at full clock. ----
            wdum = const.tile([P, 64], BF16)
            nc.vector.memset(wdum, 0.25)
            wps = ps_acc.tile([P, QBS], F32, name="warm", tag="ps_ot")

            def dummy(n):
                for _ in range(n):
                    nc.tensor.matmul(
                        wps[0:64, 0:64], lhsT=wdum, rhs=wdum[:, 0:64],
                        start=True, stop=True, skip_group_check=True,
                    )

            dummy(N_WARM)

            # ---- constants / weights into SBUF; order = first-need order ----
            wq_sb = const.tile([P, KD, DC], BF16)
            wk_sb = const.tile([P, KD, DC], BF16)
            wv_sb = const.tile([P, KD, DC], BF16)
            hT_sb = const.tile([P, NQB, KD, QBS], BF16)
            rope_sb = const.tile([P, NQB, 2, QBS], BF16)
            rot_sb = const.tile([P, P], BF16)
            cm_sb = const.tile([P, 4, QBS], BF16)
            wo_sb = const.tile([P, NHC, D], BF16)
            nc.sync.dma_start(out=wq_sb[:, 0:8, :], in_=wq[:, 0:8, :])
            nc.sync.dma_start(
                out=hT_sb[:, 0, 0:4, :], in_=hT[:, 0, 0:4, :]
            )
            nc.sync.dma_start(out=wq_sb[:, 8:16, :], in_=wq[:, 8:16, :])
            for c4 in range(1, 4):
                nc.sync.dma_start(
                    out=hT_sb[:, 0, c4 * 4:(c4 + 1) * 4, :],
                    in_=hT[:, 0, c4 * 4:(c4 + 1) * 4, :],
                )
            nc.sync.dma_start(out=wk_sb, in_=wk[:, :, :])
            nc.sync.dma_start(out=rot_sb, in_=rot[:, :])
            nc.sync.dma_start(out=rope_sb[:, 0], in_=rope[:, 0])
            nc.sync.dma_start(out=wv_sb, in_=wv[:, :, :])
            nc.sync.dma_start(out=cm_sb, in_=cmask[:, :, :])
            for qb in range(1, NQB):
                nc.sync.dma_start(out=hT_sb[:, qb], in_=hT[:, qb])
                nc.sync.dma_start(out=rope_sb[:, qb], in_=rope[:, qb])
            nc.sync.dma_start(out=wo_sb, in_=wo[:, :, :])
            ones_colb = const.tile([P, 1], BF16)   # lhsT for k-partition sums
            nc.vector.memset(ones_colb, 1.0)
            ones_row = const.tile([1, P], BF16)    # lhsT for partition broadcast
            nc.vector.memset(ones_row, 1.0)

            # persistent activations
            qt_sb = acts.tile([P, NHC, S], BF16)   # [hd, h, s] rotary-applied Q^T
            kt_sb = acts.tile([P, NHC, S], BF16)
            v_sb = acts.tile([P, NST, DC], BF16)   # [s%128, s//128, head*hd]
            otb_sb = acts.tile([P, NHC, S], BF16)  # normalized O^T per head

            # ---- projections; rope epilogues (psum->sbuf copy, rotation
            # matmul, 3 DVE ops) are deferred and spread over the V chains ----
            pending = []  # (psum, dst_sb, h, qb)

            def flush_rope(keep=0):
                while len(pending) > keep:
                    ps, dst_sb, h, qb = pending.pop(0)
                    sl = slice(qb * QBS, (qb + 1) * QBS)
                    raw = work.tile([P, QBS], BF16, tag="raw")
                    nc.scalar.activation(raw, ps, Copy)
                    rps = ps_score.tile([P, QBS], F32, tag="mm")
                    nc.tensor.matmul(rps, lhsT=rot_sb, rhs=raw, start=True, stop=True)
                    t1 = work.tile([P, QBS], BF16, tag="t1")
                    t2 = work.tile([P, QBS], BF16, tag="t2")
                    nc.vector.tensor_mul(t1, raw, rope_sb[:, qb, 0])
                    nc.vector.tensor_mul(t2, rps, rope_sb[:, qb, 1])
                    nc.vector.tensor_add(dst_sb[:, h, sl], t1, t2)

            def qk_pair(w_sb, dst_sb, qb, warm=False):
                # both heads' 16-matmul chains interleaved per-kd so chunked
                # hT DMA delivery keeps up with PE consumption
                pss = [
                    ps_score.tile([P, QBS], F32, name=f"pp{h}", tag="mm")
                    for h in range(NHC)
                ]
                for kd in range(KD):
                    for h in range(NHC):
                        nc.tensor.matmul(
                            pss[h],
                            lhsT=w_sb[:, kd, h * HD:(h + 1) * HD],
                            rhs=hT_sb[:, qb, kd, :],
                            start=(kd == 0),
                            stop=(kd == KD - 1),
                        )
                    if warm and kd in (3, 7, 11):
                        dummy(12)
                for h in range(NHC):
                    pending.append((pss[h], dst_sb, h, qb))

            def proj_block(qb, warm=False):
                qk_pair(wq_sb, qt_sb, qb, warm=warm)
                if warm:
                    dummy(10)
                qk_pair(wk_sb, kt_sb, qb)
                for s4 in range(4):
                    flush_rope(keep=3 - s4)
                    st_idx = qb * 4 + s4
                    ps = ps_op.tile([P, QBS], F32, tag="op")
                    for kd in range(KD):
                        nc.tensor.matmul(
                            ps[:, 0:DC],
                            lhsT=hT_sb[:, qb, kd, s4 * P:(s4 + 1) * P],
                            rhs=wv_sb[:, kd, :],
                            start=(kd == 0),
                            stop=(kd == KD - 1),
                        )
                    nc.scalar.activation(v_sb[:, st_idx, :], ps[:, 0:DC], Copy)
                flush_rope()

            # ---- out-projection row-groups ----
            def out_proj(qb, s4s=range(4), tail=False):
                for s4 in s4s:
                    st_idx = qb * 4 + s4
                    ost = outstage.tile([P, D], BF16, tag="ost")
                    for eb in range(NQB):
                        ops = ps_op.tile([P, QBS], F32, name="ops", tag="op")
                        for h in range(NHC):
                            nc.tensor.matmul(
                                ops,
                                lhsT=otb_sb[:, h, st_idx * P:(st_idx + 1) * P],
                                rhs=wo_sb[:, h, eb * QBS:(eb + 1) * QBS],
                                start=(h == 0),
                                stop=(h == NHC - 1),
                            )
                        osl = ost[:, eb * QBS:(eb + 1) * QBS]
                        if eb % 2 == 0:
                            nc.scalar.activation(osl, ops, Copy)
                        else:
                            nc.vector.tensor_copy(osl, ops)
                        if tail and eb == 1:
                            nc.sync.dma_start(
                                out=out[st_idx * P:(st_idx + 1) * P, 0:2 * QBS],
                                in_=ost[:, 0:2 * QBS],
                            )
                    if tail:
                        nc.sync.dma_start(
                            out=out[st_idx * P:(st_idx + 1) * P, 2 * QBS:],
                            in_=ost[:, 2 * QBS:],
                        )
                    else:
                        nc.sync.dma_start(
                            out=out[st_idx * P:(st_idx + 1) * P, :], in_=ost
                        )

            OT_LAG = 3  # P.V matmul trails the score matmul so its sem wait
            # is already satisfied and LDWEIGHTS pipelines.
            blk = {}  # per-qb state carried from kt loop to den chain

            def att_ktloop(qb, op_qb=None):
                qsl = slice(qb * QBS, (qb + 1) * QBS)
                kmax = (qb + 1) * 4
                ot_pss, pts = [], {}
                # 4-lane bf16 partial sums of exp tiles (softmax denominator);
                # all adds run in the DVE 16-bit fast mode, chains stay short.
                accs = [[None] * 4 for _ in range(NHC)]

                def acc_pt(h, kt, pt):
                    lane = kt % 4
                    if accs[h][lane] is None:
                        acc = work.tile(
                            [P, QBS], BF16, name=f"za{h}_{lane}",
                            tag=f"za{h}_{lane}",
                        )
                        nc.vector.tensor_copy(acc, pt)
                        accs[h][lane] = acc
                    else:
                        acc = accs[h][lane]
                        nc.vector.tensor_add(acc, acc, pt)

                for h in range(NHC):
                    ot_pss.append(ps_acc.tile([P, QBS], F32, name="ot_ps", tag="ps_ot"))

                kt_order = list(range(qb * 4, kmax)) + list(range(0, qb * 4))
                # spread the previous-previous block's out-proj groups through
                # this loop so their psum->sbuf copies never pile up
                op_at = {kmax // 2 - 1: 0, kmax - 3: 1} if op_qb is not None else {}

                def pv_step(kt):
                    for h in range(NHC):
                        nc.tensor.matmul(
                            ot_pss[h],
                            lhsT=v_sb[:, kt, h * HD:(h + 1) * HD],
                            rhs=pts[(h, kt)],
                            start=(kt == kt_order[0]),
                            stop=(kt == kt_order[-1]),
                            skip_group_check=True,
                        )

                for ki, kt in enumerate(kt_order):
                    for h in range(NHC):
                        sps = ps_score.tile([P, QBS], F32, tag="mm")
                        nc.tensor.matmul(
                            sps,
                            lhsT=kt_sb[:, h, kt * P:(kt + 1) * P],
                            rhs=qt_sb[:, h, qsl],
                            start=True,
                            stop=True,
                        )
                        pt = ptpool.tile([P, QBS], BF16, tag=f"pt{h}")
                        nc.scalar.activation(pt, sps, Exp, scale=SCALE)
                        j = kt - qb * 4
                        if j >= 0:  # diagonal tile: causal mask on DVE
                            nc.vector.tensor_mul(pt, pt, cm_sb[:, j, :])
                        pts[(h, kt)] = pt
                        acc_pt(h, kt, pt)
                    if ki >= OT_LAG:
                        pv_step(kt_order[ki - OT_LAG])
                    if ki in op_at:
                        out_proj(op_qb, s4s=(op_at[ki],))
                for ki2 in range(max(kmax - OT_LAG, 0), kmax):
                    pv_step(kt_order[ki2])
                blk[qb] = (ot_pss, accs)

            def att_den(qb):
                qsl = slice(qb * QBS, (qb + 1) * QBS)
                ot_pss, accs = blk.pop(qb)
                den_sbs = []
                for h in range(NHC):
                    # merge the 4 bf16 lanes on DVE, then one ones-matmul
                    lanes = [a for a in accs[h] if a is not None]
                    while len(lanes) > 1:
                        nxt = []
                        for i in range(0, len(lanes) - 1, 2):
                            nc.vector.tensor_add(lanes[i], lanes[i], lanes[i + 1])
                            nxt.append(lanes[i])
                        if len(lanes) % 2:
                            nxt.append(lanes[-1])
                        lanes = nxt
                    den_ps = ps_op.tile([P, QBS], F32, name="den_ps", tag="op")
                    nc.tensor.matmul(
                        den_ps[0:1, :], lhsT=ones_colb, rhs=lanes[0],
                        start=True, stop=True, skip_group_check=True,
                    )
                    # 1/x as exp(-ln(x)) on the scalar engine: the sanctioned
                    # DVE reciprocal runs 1 lane * 512 elems ~ 3.3us, far too
                    # slow for the tail critical path.
                    lnd = work.tile([1, QBS], F32, tag=f"ln{h}")
                    nc.scalar.activation(lnd, den_ps[0:1, :], Ln)
                    r_bf = work.tile([1, QBS], BF16, tag=f"rb{h}")
                    nc.scalar.activation(r_bf, lnd, Exp, scale=-1.0)
                    den_sbs.append(r_bf)
                for h in range(NHC):
                    bc_ps = ps_op.tile([P, QBS], F32, name="bc_ps", tag="op")
                    nc.tensor.matmul(
                        bc_ps, lhsT=ones_row, rhs=den_sbs[h], start=True, stop=True
                    )
                    bc_sb = work.tile([P, QBS], F32, tag=f"bc{h}")
                    nc.vector.tensor_copy(bc_sb, bc_ps)
                    nc.vector.tensor_mul(otb_sb[:, h, qsl], ot_pss[h], bc_sb)

            # ---- main schedule: den chains hide under later PE work ----
            proj_block(0, warm=True)
            att_ktloop(0)
            proj_block(1)
            att_den(0)
            att_ktloop(1)
            proj_block(2)
            att_den(1)
            att_ktloop(2, op_qb=0)
            out_proj(0, s4s=(2, 3))
            proj_block(3)
            att_den(2)
            att_ktloop(3, op_qb=1)
            out_proj(1, s4s=(2, 3))  # fills the PE while den(3)'s DVE runs
            att_den(3)
            out_proj(2)
            out_proj(3, tail=True)
    if split_waits:
        _split_excess_waits(nc)
    return nc


_NC_CACHE = {}


def _get_nc():
    if "nc" not in _NC_CACHE:
        _NC_CACHE["nc"] = build_nc()
    return _NC_CACHE["nc"]


def _rotation_matrix_T():
    # rot(x)[2i] = -x[2i+1]; rot(x)[2i+1] = x[2i].  R[i,j] coefficient of x[j].
    R = np.zeros((HD, HD), np.float32)
    idx = np.arange(0, HD, 2)
    R[idx, idx + 1] = -1.0
    R[idx + 1, idx] = 1.0
    return np.ascontiguousarray(R.T)


def prepare_in_maps(hidden_states, sin, cos, Wq, Wk, Wv, Wo):
    hidden_states = np.asarray(hidden_states, dtype=np.float32)
    sin = np.asarray(sin, dtype=np.float32)
    cos = np.asarray(cos, dtype=np.float32)
    Wq = np.asarray(Wq, dtype=np.float32)
    Wk = np.asarray(Wk, dtype=np.float32)
    Wv = np.asarray(Wv, dtype=np.float32)
    Wo = np.asarray(Wo, dtype=np.float32)

    hT = hidden_states[0].T.astype(NPBF16)  # [D, S]
    hT4 = np.ascontiguousarray(
        hT.reshape(KD, P, NQB, QBS).transpose(1, 2, 0, 3)
    )  # [P, NQB, KD, QBS]
    ct = np.repeat(cos, 2, axis=1).T  # [P, S]
    st = np.repeat(sin, 2, axis=1).T
    rope = np.ascontiguousarray(
        np.stack(
            [ct.reshape(P, NQB, QBS), st.reshape(P, NQB, QBS)], axis=2
        )
    ).astype(NPBF16)  # [P, NQB, 2, QBS]
    rot = _rotation_matrix_T().astype(NPBF16)
    kk, jj, xx = np.meshgrid(
        np.arange(P), np.arange(4), np.arange(QBS), indexing="ij"
    )
    cm = (xx >= jj * P + kk).astype(NPBF16)  # [P, 4, QBS]

    in_maps = []
    for c in range(N_CORES):
        e0 = c * DC
        wq_p = np.ascontiguousarray(
            Wq[e0:e0 + DC, :].T.astype(NPBF16).reshape(KD, P, DC).transpose(1, 0, 2)
        )
        wk_p = np.ascontiguousarray(
            Wk[e0:e0 + DC, :].T.astype(NPBF16).reshape(KD, P, DC).transpose(1, 0, 2)
        )
        wv_p = np.ascontiguousarray(
            Wv[e0:e0 + DC, :].T.astype(NPBF16).reshape(KD, P, DC).transpose(1, 0, 2)
        )
        wo_p = np.ascontiguousarray(
            Wo[:, e0:e0 + DC].T.astype(NPBF16).reshape(NHC, P, D).transpose(1, 0, 2)
        )
        in_maps.append(
            {
                "hT": hT4,
                "wq": wq_p,
                "wk": wk_p,
                "wv": wv_p,
                "wo": wo_p,
                "rope": rope,
                "rot": rot,
                "cmask": cm,
            }
        )
    return in_maps


def kernel(hidden_states, attention_mask, sin, cos, Wq, Wk, Wv, Wo):
    in_maps = prepare_in_maps(hidden_states, sin, cos, Wq, Wk, Wv, Wo)
    nc = _get_nc()
    res = run_bass_kernel_spmd(nc, in_maps, list(range(N_CORES)))
    out = res.results[0]["out"].astype(np.float32)
    for c in range(1, N_CORES):
        out += res.results[c]["out"].astype(np.float32)
    return out[None]


# revision 11
# speedup vs baseline: 1.1871x; 1.0446x over previous
"""GPT-J joint attention (B=1, S=2048, D=2048, H=16, HD=128) on 8 Trainium2
NeuronCores, tensor-parallel over heads (2 heads per core).

Per-core program (all matmuls bf16 inputs, fp32 PSUM accumulation):
  - QT/KT = W[qk]_shard @ hidden^T        ([hd, s] layout, per head)
  - RoPE applied via a rotation-matrix matmul + elementwise combine
  - V = hidden @ Wv_shard^T               ([s, hd] layout)
  - scores^T tiles = KT_tile^T . QT_block ([k, q] layout) -> exp -> causal
    mask via a precomputed 0/1 mask multiply on DVE
  - O^T accumulated as V_tile^T . P^T; softmax denominator via ones-matmul
  - partial out = O^T{normalized}^T . Wo_shard^T, streamed to DRAM per row-block

v3 structure: host pre-lays-out all inputs so every DMA is contiguous; a
dummy-matmul warmup chain un-throttles the PE clock (HAM) while the first
weights stream in; Q/K projection chains for both heads interleave per-kd so
chunked hT DMA keeps up; attention for block qb runs right after its
projections; each block's softmax-denominator chain is emitted after later
independent PE work so the PE never head-of-line blocks on DVE; out-proj
row-groups are spread through the next attention loops to smooth the
psum->sbuf copy load.

Host side: shard/transpose/cast inputs, run SPMD on 8 cores, sum the 8
partial outputs (the tensor-parallel all-reduce equivalent).
"""
import sys

import numpy as np
import ml_dtypes

try:
    import concourse.bass as bass
except ImportError:  # pragma: no cover
    sys.path.insert(0, "/opt/trn_rl_repo")
    import concourse.bass as bass

import concourse.mybir as mybir
import concourse.tile as tile
from concourse.bass_utils import run_bass_kernel_spmd

BF16 = mybir.dt.bfloat16
F32 = mybir.dt.float32
NPBF16 = ml_dtypes.bfloat16

N_CORES = 8
S = 2048          # sequence length
D = 2048          # model dim
HD = 128          # head dim
NHC = 2           # heads per core
DC = NHC * HD     # shard width (256)
P = 128           # partitions
KD = D // P       # 16 contraction tiles over model dim
QBS = 512         # q-block size
NQB = S // QBS    # 4 q-blocks
NST = S // P      # 16 sequence tiles of 128
SCALE = 1.0 / float(np.sqrt(HD))
N_WARM = 60       # dummy matmuls covering the pre-DMA window (~55ns each cold)

# ---------------------------------------------------------------------------
# Walrus's CoreV3 drain encoding accepts a single sem wait; Tile's tail drain
# carries one wait per logical proc. Split it into one drain per proc.
# ---------------------------------------------------------------------------


def _install_drain_split():
    if getattr(tile.TileContext, "_drain_split_installed", False):
        return
    from concourse.vector_clock import ScopedClock, VectorClock

    def _drain_and_barrier(self, tick_clock, wait_clock):
        full = tick_clock.global_clock
        n = len(full)
        for i in range(n):
            if full[i] <= 0:
                continue
            vec = [full[j] if j == i else 0 for j in range(n)]
            drain_inst = self.nc.sync.drain()
            wait_clock.add_sem_waits(
                drain_inst.ins, ScopedClock({None: VectorClock(vec)})
            )
        self.nc.all_engine_barrier()
        assert self.sems is not None
        popped = self.nc._tile_sem_poison_stack.pop()
        assert popped is self._sem_poison
        self.nc.clear_and_free_semaphores(list(self.sems.allocated().values()))
        self.nc.all_engine_barrier()

    tile.TileContext._drain_and_barrier = _drain_and_barrier
    tile.TileContext._drain_split_installed = True


def _split_excess_waits(nc, limit=1):
    """This walrus build rejects instructions carrying more than one sem wait
    (CoreV3 setupSyncWait: 'Too many sync wait commands'). Spill excess waits
    onto same-engine NOPs inserted just before the instruction — the engine
    executes them in queue order, so blocking semantics are unchanged."""
    ctr = 0
    for fn in nc.m.functions:
        for blk in fn.blocks:
            new_list = []
            for inst in blk.instructions:
                si = inst.sync_info
                if si is not None and len(si.on_wait) > limit:
                    waits = list(si.on_wait)
                    excess, keep = waits[:-limit], waits[-limit:]
                    for w in excess:
                        ctr += 1
                        nop = mybir.InstNoOp(
                            name=f"I-wsplit-{ctr}", text_hint="wait_split"
                        )
                        nop.engine = inst.engine
                        nop.sync_info = mybir.SyncInfo(on_wait=[w], on_update=[])
                        new_list.append(nop)
                    inst.sync_info = mybir.SyncInfo(
                        on_wait=keep, on_update=si.on_update
                    )
                new_list.append(inst)
            if len(new_list) != len(blk.instructions):
                blk.instructions[:] = new_list
    return ctr


def build_nc(split_waits=True):
    _install_drain_split()
    nc = bass.Bass()

    # All inputs are host-pre-laid-out so each DMA below is a contiguous copy.
    hT = nc.dram_tensor("hT", [P, NQB, KD, QBS], BF16, kind="ExternalInput")
    wq = nc.dram_tensor("wq", [P, KD, DC], BF16, kind="ExternalInput")
    wk = nc.dram_tensor("wk", [P, KD, DC], BF16, kind="ExternalInput")
    wv = nc.dram_tensor("wv", [P, KD, DC], BF16, kind="ExternalInput")
    wo = nc.dram_tensor("wo", [P, NHC, D], BF16, kind="ExternalInput")
    # rope[p, qb, 0, :] = cos row p of q-block qb; rope[p, qb, 1, :] = sin
    rope = nc.dram_tensor("rope", [P, NQB, 2, QBS], BF16, kind="ExternalInput")
    rot = nc.dram_tensor("rot", [P, P], BF16, kind="ExternalInput")
    # cmask[k, j, x] = 1.0 if x >= j*128 + k else 0 (causal mask, diag tile j)
    cmask = nc.dram_tensor("cmask", [P, 4, QBS], BF16, kind="ExternalInput")
    out = nc.dram_tensor("out", [S, D], BF16, kind="ExternalOutput")

    Exp = mybir.ActivationFunctionType.Exp
    Copy = mybir.ActivationFunctionType.Copy
    Ln = mybir.ActivationFunctionType.Ln

    with tile.TileContext(nc) as tc:
        with (
            tc.tile_pool(name="const", bufs=1) as const,
            tc.tile_pool(name="acts", bufs=1) as acts,
            tc.tile_pool(name="work", bufs=2) as work,
            tc.tile_pool(name="ptpool", bufs=6) as ptpool,
            tc.tile_pool(name="outstage", bufs=3) as outstage,
            tc.tile_pool(name="ps_score", bufs=4, space="PSUM") as ps_score,
            tc.tile_pool(name="ps_op", bufs=2, space="PSUM") as ps_op,
            tc.tile_pool(name="ps_acc", bufs=2, space="PSUM") as ps_acc,
        ):
            # ---- PE warmup: HAM un-throttles after ~3.4us of sustained PE
            # activity; run a dummy matmul chain while the first DMAs land so
            # the real projection stream starts at full clock. ----
            wdum = const.tile([P, 64], BF16)
            nc.vector.memset(wdum, 0.25)
            wps = ps_acc.tile([P, QBS], F32, name="warm", tag="ps_ot")

            def dummy(n):
                for _ in range(n):
                    nc.tensor.matmul(
                        wps[0:64, 0:64], lhsT=wdum, rhs=wdum[:, 0:64],
                        start=True, stop=True, skip_group_check=True,
                    )

            dummy(N_WARM)

            # ---- constants / weights into SBUF; order = first-need order ----
            wq_sb = const.tile([P, KD, DC], BF16)
            wk_sb = const.tile([P, KD, DC], BF16)
            wv_sb = const.tile([P, KD, DC], BF16)
            hT_sb = const.tile([P, NQB, KD, QBS], BF16)
            rope_sb = const.tile([P, NQB, 2, QBS], BF16)
            rot_sb = const.tile([P, P], BF16)
            cm_sb = const.tile([P, 4, QBS], BF16)
            wo_sb = const.tile([P, NHC, D], BF16)
            nc.sync.dma_start(out=wq_sb[:, 0:8, :], in_=wq[:, 0:8, :])
            nc.sync.dma_start(out=hT_sb[:, 0, 0:4, :], in_=hT[:, 0, 0:4, :])
            nc.sync.dma_start(out=hT_sb[:, 0, 4:8, :], in_=hT[:, 0, 4:8, :])
            nc.sync.dma_start(out=wq_sb[:, 8:16, :], in_=wq[:, 8:16, :])
            for c4 in range(2, 4):
                nc.sync.dma_start(
                    out=hT_sb[:, 0, c4 * 4:(c4 + 1) * 4, :],
                    in_=hT[:, 0, c4 * 4:(c4 + 1) * 4, :],
                )
            nc.sync.dma_start(out=wk_sb, in_=wk[:, :, :])
            nc.sync.dma_start(out=rot_sb, in_=rot[:, :])
            nc.sync.dma_start(out=rope_sb[:, 0], in_=rope[:, 0])
            nc.sync.dma_start(out=wv_sb, in_=wv[:, :, :])
            nc.sync.dma_start(out=cm_sb, in_=cmask[:, :, :])
            for qb in range(1, NQB):
                nc.sync.dma_start(out=hT_sb[:, qb], in_=hT[:, qb])
                nc.sync.dma_start(out=rope_sb[:, qb], in_=rope[:, qb])
            nc.sync.dma_start(out=wo_sb, in_=wo[:, :, :])
            ones_colb = const.tile([P, 1], BF16)   # lhsT for k-partition sums
            nc.vector.memset(ones_colb, 1.0)
            ones_row = const.tile([1, P], BF16)    # lhsT for partition broadcast
            nc.vector.memset(ones_row, 1.0)

            # persistent activations
            qt_sb = acts.tile([P, NHC, S], BF16)   # [hd, h, s] rotary-applied Q^T
            kt_sb = acts.tile([P, NHC, S], BF16)
            v_sb = acts.tile([P, NST, DC], BF16)   # [s%128, s//128, head*hd]
            otb_sb = acts.tile([P, NHC, S], BF16)  # normalized O^T per head

            # ---- projections; rope epilogues (psum->sbuf copy, rotation
            # matmul, 3 DVE ops) are deferred and spread over the V chains ----
            pending = []  # (psum, dst_sb, h, qb)
            flush_ctr = [0]

            def flush_rope(keep=0):
                while len(pending) > keep:
                    ps, dst_sb, h, qb = pending.pop(0)
                    sl = slice(qb * QBS, (qb + 1) * QBS)
                    raw = work.tile([P, QBS], BF16, tag="raw")
                    # alternate the psum->sbuf copy between ACT and DVE so
                    # back-to-back flushes don't serialize on one engine
                    if flush_ctr[0] % 2 == 0:
                        nc.scalar.activation(raw, ps, Copy)
                    else:
                        nc.vector.tensor_copy(raw, ps)
                    flush_ctr[0] += 1
                    rps = ps_score.tile([P, QBS], F32, tag="mm")
                    nc.tensor.matmul(rps, lhsT=rot_sb, rhs=raw, start=True, stop=True)
                    t1 = work.tile([P, QBS], BF16, tag="t1")
                    t2 = work.tile([P, QBS], BF16, tag="t2")
                    nc.vector.tensor_mul(t1, raw, rope_sb[:, qb, 0])
                    nc.vector.tensor_mul(t2, rps, rope_sb[:, qb, 1])
                    nc.vector.tensor_add(dst_sb[:, h, sl], t1, t2)

            def qk_pair(w_sb, dst_sb, qb, warm=False):
                # both heads' 16-matmul chains interleaved per-kd so chunked
                # hT DMA delivery keeps up with PE consumption
                pss = [
                    ps_score.tile([P, QBS], F32, name=f"pp{h}", tag="mm")
                    for h in range(NHC)
                ]
                for kd in range(KD):
                    for h in range(NHC):
                        nc.tensor.matmul(
                            pss[h],
                            lhsT=w_sb[:, kd, h * HD:(h + 1) * HD],
                            rhs=hT_sb[:, qb, kd, :],
                            start=(kd == 0),
                            stop=(kd == KD - 1),
                        )
                    if warm and kd in (3, 7, 11):
                        dummy(12)
                for h in range(NHC):
                    pending.append((pss[h], dst_sb, h, qb))

            def proj_block(qb, warm=False):
                qk_pair(wq_sb, qt_sb, qb, warm=warm)
                if warm:
                    dummy(10)
                qk_pair(wk_sb, kt_sb, qb)
                for s4 in range(4):
                    # flush all rope epilogues by s4=2 so the attention loop's
                    # first (diagonal) score matmul never waits on kt_sb
                    flush_rope(keep=(3, 1, 0, 0)[s4])
                    st_idx = qb * 4 + s4
                    ps = ps_op.tile([P, QBS], F32, tag="op")
                    for kd in range(KD):
                        nc.tensor.matmul(
                            ps[:, 0:DC],
                            lhsT=hT_sb[:, qb, kd, s4 * P:(s4 + 1) * P],
                            rhs=wv_sb[:, kd, :],
                            start=(kd == 0),
                            stop=(kd == KD - 1),
                        )
                    nc.scalar.activation(v_sb[:, st_idx, :], ps[:, 0:DC], Copy)
                flush_rope()

            # ---- out-projection row-groups; psum->sbuf copies split in half
            # across ACT and DVE so the psum slot frees in ~350ns ----
            def out_proj(qb, s4s=range(4), tail=False, pool=None):
                pool = pool or ps_op
                ptag = "op" if pool is ps_op else "mm"
                for s4 in s4s:
                    st_idx = qb * 4 + s4
                    ost = outstage.tile([P, D], BF16, tag="ost")
                    for eb in range(NQB):
                        ops = pool.tile([P, QBS], F32, name="ops", tag=ptag)
                        for h in range(NHC):
                            nc.tensor.matmul(
                                ops,
                                lhsT=otb_sb[:, h, st_idx * P:(st_idx + 1) * P],
                                rhs=wo_sb[:, h, eb * QBS:(eb + 1) * QBS],
                                start=(h == 0),
                                stop=(h == NHC - 1),
                            )
                        osl = ost[:, eb * QBS:(eb + 1) * QBS]
                        nc.scalar.activation(osl[:, 0:QBS // 2], ops[:, 0:QBS // 2], Copy)
                        nc.vector.tensor_copy(osl[:, QBS // 2:], ops[:, QBS // 2:])
                        if tail and eb == 1:
                            nc.sync.dma_start(
                                out=out[st_idx * P:(st_idx + 1) * P, 0:2 * QBS],
                                in_=ost[:, 0:2 * QBS],
                            )
                    if tail:
                        nc.sync.dma_start(
                            out=out[st_idx * P:(st_idx + 1) * P, 2 * QBS:],
                            in_=ost[:, 2 * QBS:],
                        )
                    else:
                        nc.sync.dma_start(
                            out=out[st_idx * P:(st_idx + 1) * P, :], in_=ost
                        )

            OT_LAG = 3  # P.V matmul trails the score matmul so its sem wait
            # is already satisfied and LDWEIGHTS pipelines.
            blk = {}  # per-qb state carried from kt loop to den chain

            def att_ktloop(qb, op_qb=None):
                qsl = slice(qb * QBS, (qb + 1) * QBS)
                kmax = (qb + 1) * 4
                ot_pss, pts = [], {}
                # 4-lane bf16 partial sums of exp tiles (softmax denominator);
                # all adds run in the DVE 16-bit fast mode, chains stay short.
                accs = [[None] * 4 for _ in range(NHC)]

                def acc_pt(h, kt, pt):
                    lane = kt % 4
                    if accs[h][lane] is None:
                        acc = work.tile(
                            [P, QBS], BF16, name=f"za{h}_{lane}",
                            tag=f"za{h}_{lane}",
                        )
                        nc.vector.tensor_copy(acc, pt)
                        accs[h][lane] = acc
                    else:
                        acc = accs[h][lane]
                        nc.vector.tensor_add(acc, acc, pt)

                for h in range(NHC):
                    ot_pss.append(ps_acc.tile([P, QBS], F32, name="ot_ps", tag="ps_ot"))

                kt_order = list(range(qb * 4, kmax)) + list(range(0, qb * 4))
                # spread the previous-previous block's out-proj groups through
                # this loop so their psum->sbuf copies never pile up
                op_at = {kmax // 2 - 1: 0, kmax - 3: 1} if op_qb is not None else {}

                def pv_step(kt):
                    for h in range(NHC):
                        nc.tensor.matmul(
                            ot_pss[h],
                            lhsT=v_sb[:, kt, h * HD:(h + 1) * HD],
                            rhs=pts[(h, kt)],
                            start=(kt == kt_order[0]),
                            stop=(kt == kt_order[-1]),
                            skip_group_check=True,
                        )

                for ki, kt in enumerate(kt_order):
                    for h in range(NHC):
                        sps = ps_score.tile([P, QBS], F32, tag="mm")
                        nc.tensor.matmul(
                            sps,
                            lhsT=kt_sb[:, h, kt * P:(kt + 1) * P],
                            rhs=qt_sb[:, h, qsl],
                            start=True,
                            stop=True,
                        )
                        pt = ptpool.tile([P, QBS], BF16, tag=f"pt{h}")
                        nc.scalar.activation(pt, sps, Exp, scale=SCALE)
                        j = kt - qb * 4
                        if j >= 0:  # diagonal tile: causal mask on DVE
                            nc.vector.tensor_mul(pt, pt, cm_sb[:, j, :])
                        pts[(h, kt)] = pt
                        acc_pt(h, kt, pt)
                    if ki >= OT_LAG:
                        pv_step(kt_order[ki - OT_LAG])
                    if ki in op_at:
                        out_proj(op_qb, s4s=(op_at[ki],))
                for ki2 in range(max(kmax - OT_LAG, 0), kmax):
                    pv_step(kt_order[ki2])
                blk[qb] = (ot_pss, accs)

            def att_den(qb):
                qsl = slice(qb * QBS, (qb + 1) * QBS)
                ot_pss, accs = blk.pop(qb)
                den_sbs = []
                for h in range(NHC):
                    # merge the 4 bf16 lanes on DVE, then one ones-matmul
                    lanes = [a for a in accs[h] if a is not None]
                    while len(lanes) > 1:
                        nxt = []
                        for i in range(0, len(lanes) - 1, 2):
                            nc.vector.tensor_add(lanes[i], lanes[i], lanes[i + 1])
                            nxt.append(lanes[i])
                        if len(lanes) % 2:
                            nxt.append(lanes[-1])
                        lanes = nxt
                    den_ps = ps_op.tile([P, QBS], F32, name="den_ps", tag="op")
                    nc.tensor.matmul(
                        den_ps[0:1, :], lhsT=ones_colb, rhs=lanes[0],
                        start=True, stop=True, skip_group_check=True,
                    )
                    # 1/x as exp(-ln(x)) on the scalar engine: the sanctioned
                    # DVE reciprocal runs 1 lane * 512 elems ~ 3.3us, far too
                    # slow for the tail critical path.
                    lnd = work.tile([1, QBS], F32, tag=f"ln{h}")
                    nc.scalar.activation(lnd, den_ps[0:1, :], Ln)
                    r_bf = work.tile([1, QBS], BF16, tag=f"rb{h}")
                    nc.scalar.activation(r_bf, lnd, Exp, scale=-1.0)
                    den_sbs.append(r_bf)
                for h in range(NHC):
                    bc_ps = ps_op.tile([P, QBS], F32, name="bc_ps", tag="op")
                    nc.tensor.matmul(
                        bc_ps, lhsT=ones_row, rhs=den_sbs[h], start=True, stop=True
                    )
                    bc_sb = work.tile([P, QBS], F32, tag=f"bc{h}")
                    nc.vector.tensor_copy(bc_sb, bc_ps)
                    nc.vector.tensor_mul(otb_sb[:, h, qsl], ot_pss[h], bc_sb)

            # ---- main schedule: den chains hide under later PE work ----
            proj_block(0, warm=True)
            att_ktloop(0)
            proj_block(1)
            att_den(0)
            att_ktloop(1)
            proj_block(2)
            att_den(1)
            att_ktloop(2, op_qb=0)
            out_proj(0, s4s=(2, 3))
            proj_block(3)
            att_den(2)
            att_ktloop(3, op_qb=1)
            # tail: scores are done, so the 4-slot score psum pool is free —
            # use it for the final out-projections to avoid 2-slot stalls
            out_proj(1, s4s=(2, 3), pool=ps_score)  # fills PE under den(3) DVE
            att_den(3)
            out_proj(2, pool=ps_score)
            out_proj(3, tail=True, pool=ps_score)
    if split_waits:
        _split_excess_waits(nc)
    return nc


_NC_CACHE = {}


def _get_nc():
    if "nc" not in _NC_CACHE:
        _NC_CACHE["nc"] = build_nc()
    return _NC_CACHE["nc"]


def _rotation_matrix_T():
    # rot(x)[2i] = -x[2i+1]; rot(x)[2i+1] = x[2i].  R[i,j] coefficient of x[j].
    R = np.zeros((HD, HD), np.float32)
    idx = np.arange(0, HD, 2)
    R[idx, idx + 1] = -1.0
    R[idx + 1, idx] = 1.0
    return np.ascontiguousarray(R.T)


def prepare_in_maps(hidden_states, sin, cos, Wq, Wk, Wv, Wo):
    hidden_states = np.asarray(hidden_states, dtype=np.float32)
    sin = np.asarray(sin, dtype=np.float32)
    cos = np.asarray(cos, dtype=np.float32)
    Wq = np.asarray(Wq, dtype=np.float32)
    Wk = np.asarray(Wk, dtype=np.float32)
    Wv = np.asarray(Wv, dtype=np.float32)
    Wo = np.asarray(Wo, dtype=np.float32)

    hT = hidden_states[0].T.astype(NPBF16)  # [D, S]
    hT4 = np.ascontiguousarray(
        hT.reshape(KD, P, NQB, QBS).transpose(1, 2, 0, 3)
    )  # [P, NQB, KD, QBS]
    ct = np.repeat(cos, 2, axis=1).T  # [P, S]
    st = np.repeat(sin, 2, axis=1).T
    rope = np.ascontiguousarray(
        np.stack(
            [ct.reshape(P, NQB, QBS), st.reshape(P, NQB, QBS)], axis=2
        )
    ).astype(NPBF16)  # [P, NQB, 2, QBS]
    rot = _rotation_matrix_T().astype(NPBF16)
    kk, jj, xx = np.meshgrid(
        np.arange(P), np.arange(4), np.arange(QBS), indexing="ij"
    )
    cm = (xx >= jj * P + kk).astype(NPBF16)  # [P, 4, QBS]

    in_maps = []
    for c in range(N_CORES):
        e0 = c * DC
        wq_p = np.ascontiguousarray(
            Wq[e0:e0 + DC, :].T.astype(NPBF16).reshape(KD, P, DC).transpose(1, 0, 2)
        )
        wk_p = np.ascontiguousarray(
            Wk[e0:e0 + DC, :].T.astype(NPBF16).reshape(KD, P, DC).transpose(1, 0, 2)
        )
        wv_p = np.ascontiguousarray(
            Wv[e0:e0 + DC, :].T.astype(NPBF16).reshape(KD, P, DC).transpose(1, 0, 2)
        )
        wo_p = np.ascontiguousarray(
            Wo[:, e0:e0 + DC].T.astype(NPBF16).reshape(NHC, P, D).transpose(1, 0, 2)
        )
        in_maps.append(
            {
                "hT": hT4,
                "wq": wq_p,
                "wk": wk_p,
                "wv": wv_p,
                "wo": wo_p,
                "rope": rope,
                "rot": rot,
                "cmask": cm,
            }
        )
    return in_maps


def kernel(hidden_states, attention_mask, sin, cos, Wq, Wk, Wv, Wo):
    in_maps = prepare_in_maps(hidden_states, sin, cos, Wq, Wk, Wv, Wo)
    nc = _get_nc()
    res = run_bass_kernel_spmd(nc, in_maps, list(range(N_CORES)))
    out = res.results[0]["out"].astype(np.float32)
    for c in range(1, N_CORES):
        out += res.results[c]["out"].astype(np.float32)
    return out[None]


# revision 13
# speedup vs baseline: 1.2099x; 1.0193x over previous
"""GPT-J joint attention (B=1, S=2048, D=2048, H=16, HD=128) on 8 Trainium2
NeuronCores, tensor-parallel over heads (2 heads per core).

Per-core program (all matmuls bf16 inputs, fp32 PSUM accumulation):
  - QT/KT = W[qk]_shard @ hidden^T        ([hd, s] layout, per head)
  - RoPE applied via a rotation-matrix matmul + elementwise combine
  - V = hidden @ Wv_shard^T               ([s, hd] layout)
  - scores^T tiles = KT_tile^T . QT_block ([k, q] layout) -> exp -> causal
    mask via a precomputed 0/1 mask multiply on DVE
  - O^T accumulated as V_tile^T . P^T; softmax denominator via ones-matmul
  - partial out = O^T{normalized}^T . Wo_shard^T, streamed to DRAM per row-block

v3 structure: host pre-lays-out all inputs so every DMA is contiguous; a
dummy-matmul warmup chain un-throttles the PE clock (HAM) while the first
weights stream in; Q/K projection chains for both heads interleave per-kd so
chunked hT DMA keeps up; attention for block qb runs right after its
projections; each block's softmax-denominator chain is emitted after later
independent PE work so the PE never head-of-line blocks on DVE; out-proj
row-groups are spread through the next attention loops to smooth the
psum->sbuf copy load.

Host side: shard/transpose/cast inputs, run SPMD on 8 cores, sum the 8
partial outputs (the tensor-parallel all-reduce equivalent).
"""
import sys

import numpy as np
import ml_dtypes

try:
    import concourse.bass as bass
except ImportError:  # pragma: no cover
    sys.path.insert(0, "/opt/trn_rl_repo")
    import concourse.bass as bass

import concourse.mybir as mybir
import concourse.tile as tile
from concourse.bass_utils import run_bass_kernel_spmd

BF16 = mybir.dt.bfloat16
F32 = mybir.dt.float32
NPBF16 = ml_dtypes.bfloat16

N_CORES = 8
S = 2048          # sequence length
D = 2048          # model dim
HD = 128          # head dim
NHC = 2           # heads per core
DC = NHC * HD     # shard width (256)
P = 128           # partitions
KD = D // P       # 16 contraction tiles over model dim
QBS = 512         # q-block size
NQB = S // QBS    # 4 q-blocks
NST = S // P      # 16 sequence tiles of 128
SCALE = 1.0 / float(np.sqrt(HD))
N_WARM = 60       # dummy matmuls covering the pre-DMA window (~55ns each cold)

# ---------------------------------------------------------------------------
# Walrus's CoreV3 drain encoding accepts a single sem wait; Tile's tail drain
# carries one wait per logical proc. Split it into one drain per proc.
# ---------------------------------------------------------------------------


def _install_drain_split():
    if getattr(tile.TileContext, "_drain_split_installed", False):
        return
    from concourse.vector_clock import ScopedClock, VectorClock

    def _drain_and_barrier(self, tick_clock, wait_clock):
        full = tick_clock.global_clock
        n = len(full)
        for i in range(n):
            if full[i] <= 0:
                continue
            vec = [full[j] if j == i else 0 for j in range(n)]
            drain_inst = self.nc.sync.drain()
            wait_clock.add_sem_waits(
                drain_inst.ins, ScopedClock({None: VectorClock(vec)})
            )
        self.nc.all_engine_barrier()
        assert self.sems is not None
        popped = self.nc._tile_sem_poison_stack.pop()
        assert popped is self._sem_poison
        self.nc.clear_and_free_semaphores(list(self.sems.allocated().values()))
        self.nc.all_engine_barrier()

    tile.TileContext._drain_and_barrier = _drain_and_barrier
    tile.TileContext._drain_split_installed = True


def _split_excess_waits(nc, limit=1):
    """This walrus build rejects instructions carrying more than one sem wait
    (CoreV3 setupSyncWait: 'Too many sync wait commands'). Spill excess waits
    onto same-engine NOPs inserted just before the instruction — the engine
    executes them in queue order, so blocking semantics are unchanged."""
    ctr = 0
    for fn in nc.m.functions:
        for blk in fn.blocks:
            new_list = []
            for inst in blk.instructions:
                si = inst.sync_info
                if si is not None and len(si.on_wait) > limit:
                    waits = list(si.on_wait)
                    excess, keep = waits[:-limit], waits[-limit:]
                    for w in excess:
                        ctr += 1
                        nop = mybir.InstNoOp(
                            name=f"I-wsplit-{ctr}", text_hint="wait_split"
                        )
                        nop.engine = inst.engine
                        nop.sync_info = mybir.SyncInfo(on_wait=[w], on_update=[])
                        new_list.append(nop)
                    inst.sync_info = mybir.SyncInfo(
                        on_wait=keep, on_update=si.on_update
                    )
                new_list.append(inst)
            if len(new_list) != len(blk.instructions):
                blk.instructions[:] = new_list
    return ctr


def build_nc(split_waits=True):
    _install_drain_split()
    nc = bass.Bass()

    # All inputs are host-pre-laid-out so each DMA below is a contiguous copy.
    hT = nc.dram_tensor("hT", [P, NQB, KD, QBS], BF16, kind="ExternalInput")
    wq = nc.dram_tensor("wq", [P, KD, DC], BF16, kind="ExternalInput")
    wk = nc.dram_tensor("wk", [P, KD, DC], BF16, kind="ExternalInput")
    wv = nc.dram_tensor("wv", [P, KD, DC], BF16, kind="ExternalInput")
    wo = nc.dram_tensor("wo", [P, NHC, D], BF16, kind="ExternalInput")
    # rope[p, qb, 0, :] = cos row p of q-block qb; rope[p, qb, 1, :] = sin
    rope = nc.dram_tensor("rope", [P, NQB, 2, QBS], BF16, kind="ExternalInput")
    rot = nc.dram_tensor("rot", [P, P], BF16, kind="ExternalInput")
    # cmask[k, j, x] = 1.0 if x >= j*128 + k else 0 (causal mask, diag tile j)
    cmask = nc.dram_tensor("cmask", [P, 4, QBS], BF16, kind="ExternalInput")
    out = nc.dram_tensor("out", [S, D], BF16, kind="ExternalOutput")

    Exp = mybir.ActivationFunctionType.Exp
    Copy = mybir.ActivationFunctionType.Copy
    Ln = mybir.ActivationFunctionType.Ln

    with tile.TileContext(nc) as tc:
        with (
            tc.tile_pool(name="const", bufs=1) as const,
            tc.tile_pool(name="acts", bufs=1) as acts,
            tc.tile_pool(name="work", bufs=2) as work,
            tc.tile_pool(name="ptpool", bufs=6) as ptpool,
            tc.tile_pool(name="outstage", bufs=3) as outstage,
            tc.tile_pool(name="ps_score", bufs=4, space="PSUM") as ps_score,
            tc.tile_pool(name="ps_op", bufs=2, space="PSUM") as ps_op,
            tc.tile_pool(name="ps_acc", bufs=2, space="PSUM") as ps_acc,
        ):
            # ---- PE warmup: HAM un-throttles after ~3.4us of sustained PE
            # activity; run a dummy matmul chain while the first DMAs land so
            # the real projection stream starts at full clock. ----
            wdum = const.tile([P, 64], BF16)
            nc.vector.memset(wdum, 0.25)
            wps = ps_acc.tile([P, QBS], F32, name="warm", tag="ps_ot")

            def dummy(n):
                for _ in range(n):
                    nc.tensor.matmul(
                        wps[0:64, 0:64], lhsT=wdum, rhs=wdum[:, 0:64],
                        start=True, stop=True, skip_group_check=True,
                    )

            dummy(N_WARM)

            # ---- constants / weights into SBUF; order = first-need order ----
            wq_sb = const.tile([P, KD, DC], BF16)
            wk_sb = const.tile([P, KD, DC], BF16)
            wv_sb = const.tile([P, KD, DC], BF16)
            hT_sb = const.tile([P, NQB, KD, QBS], BF16)
            rope_sb = const.tile([P, NQB, 2, QBS], BF16)
            rot_sb = const.tile([P, P], BF16)
            cm_sb = const.tile([P, 4, QBS], BF16)
            wo_sb = const.tile([P, NHC, D], BF16)
            nc.sync.dma_start(out=wq_sb[:, 0:8, :], in_=wq[:, 0:8, :])
            nc.sync.dma_start(out=hT_sb[:, 0, 0:4, :], in_=hT[:, 0, 0:4, :])
            nc.sync.dma_start(out=hT_sb[:, 0, 4:8, :], in_=hT[:, 0, 4:8, :])
            nc.sync.dma_start(out=wq_sb[:, 8:16, :], in_=wq[:, 8:16, :])
            for c4 in range(2, 4):
                nc.sync.dma_start(
                    out=hT_sb[:, 0, c4 * 4:(c4 + 1) * 4, :],
                    in_=hT[:, 0, c4 * 4:(c4 + 1) * 4, :],
                )
            nc.sync.dma_start(out=wk_sb, in_=wk[:, :, :])
            nc.sync.dma_start(out=rot_sb, in_=rot[:, :])
            nc.sync.dma_start(out=rope_sb[:, 0], in_=rope[:, 0])
            nc.sync.dma_start(out=wv_sb, in_=wv[:, :, :])
            nc.sync.dma_start(out=cm_sb, in_=cmask[:, :, :])
            for qb in range(1, NQB):
                nc.sync.dma_start(out=hT_sb[:, qb], in_=hT[:, qb])
                nc.sync.dma_start(out=rope_sb[:, qb], in_=rope[:, qb])
            nc.sync.dma_start(out=wo_sb, in_=wo[:, :, :])
            ones_colb = const.tile([P, 1], BF16)   # lhsT for k-partition sums
            nc.vector.memset(ones_colb, 1.0)
            ones_row = const.tile([1, P], BF16)    # lhsT for partition broadcast
            nc.vector.memset(ones_row, 1.0)

            # persistent activations
            qt_sb = acts.tile([P, NHC, S], BF16)   # [hd, h, s] rotary-applied Q^T
            kt_sb = acts.tile([P, NHC, S], BF16)
            v_sb = acts.tile([P, NST, DC], BF16)   # [s%128, s//128, head*hd]
            otb_sb = acts.tile([P, NHC, S], BF16)  # normalized O^T per head

            # ---- projections; rope epilogues (psum->sbuf copy, rotation
            # matmul, 3 DVE ops) are deferred and spread over the V chains ----
            pending = []  # (psum, dst_sb, h, qb)
            flush_ctr = [0]

            def flush_rope(keep=0):
                while len(pending) > keep:
                    ps, dst_sb, h, qb = pending.pop(0)
                    sl = slice(qb * QBS, (qb + 1) * QBS)
                    raw = work.tile([P, QBS], BF16, tag="raw")
                    # alternate the psum->sbuf copy between ACT and DVE so
                    # back-to-back flushes don't serialize on one engine
                    if flush_ctr[0] % 2 == 0:
                        nc.scalar.activation(raw, ps, Copy)
                    else:
                        nc.vector.tensor_copy(raw, ps)
                    flush_ctr[0] += 1
                    rps = ps_score.tile([P, QBS], F32, tag="mm")
                    nc.tensor.matmul(rps, lhsT=rot_sb, rhs=raw, start=True, stop=True)
                    t1 = work.tile([P, QBS], BF16, tag="t1")
                    t2 = work.tile([P, QBS], BF16, tag="t2")
                    nc.vector.tensor_mul(t1, raw, rope_sb[:, qb, 0])
                    nc.vector.tensor_mul(t2, rps, rope_sb[:, qb, 1])
                    nc.vector.tensor_add(dst_sb[:, h, sl], t1, t2)

            def qk_pair(w_sb, dst_sb, qb, warm=False):
                # both heads' 16-matmul chains interleaved per-kd so chunked
                # hT DMA delivery keeps up with PE consumption
                pss = [
                    ps_score.tile([P, QBS], F32, name=f"pp{h}", tag="mm")
                    for h in range(NHC)
                ]
                for kd in range(KD):
                    for h in range(NHC):
                        nc.tensor.matmul(
                            pss[h],
                            lhsT=w_sb[:, kd, h * HD:(h + 1) * HD],
                            rhs=hT_sb[:, qb, kd, :],
                            start=(kd == 0),
                            stop=(kd == KD - 1),
                        )
                    if warm and kd in (3, 7, 11):
                        dummy(12)
                for h in range(NHC):
                    pending.append((pss[h], dst_sb, h, qb))

            def proj_block(qb, warm=False):
                qk_pair(wq_sb, qt_sb, qb, warm=warm)
                if warm:
                    dummy(10)
                qk_pair(wk_sb, kt_sb, qb)
                for s4 in range(4):
                    # flush all rope epilogues by s4=2 so the attention loop's
                    # first (diagonal) score matmul never waits on kt_sb
                    flush_rope(keep=(3, 1, 0, 0)[s4])
                    st_idx = qb * 4 + s4
                    ps = ps_op.tile([P, QBS], F32, tag="op")
                    for kd in range(KD):
                        nc.tensor.matmul(
                            ps[:, 0:DC],
                            lhsT=hT_sb[:, qb, kd, s4 * P:(s4 + 1) * P],
                            rhs=wv_sb[:, kd, :],
                            start=(kd == 0),
                            stop=(kd == KD - 1),
                        )
                    nc.scalar.activation(v_sb[:, st_idx, :], ps[:, 0:DC], Copy)
                flush_rope()

            # ---- out-projection row-groups; psum->sbuf copies split in half
            # across ACT and DVE so the psum slot frees in ~350ns ----
            def out_proj(qb, s4s=range(4), tail=False, pool=None):
                pool = pool or ps_op
                ptag = "op" if pool is ps_op else "mm"
                for s4 in s4s:
                    st_idx = qb * 4 + s4
                    ost = outstage.tile([P, D], BF16, tag="ost")
                    for eb in range(NQB):
                        ops = pool.tile([P, QBS], F32, name="ops", tag=ptag)
                        for h in range(NHC):
                            nc.tensor.matmul(
                                ops,
                                lhsT=otb_sb[:, h, st_idx * P:(st_idx + 1) * P],
                                rhs=wo_sb[:, h, eb * QBS:(eb + 1) * QBS],
                                start=(h == 0),
                                stop=(h == NHC - 1),
                            )
                        osl = ost[:, eb * QBS:(eb + 1) * QBS]
                        nc.scalar.activation(osl[:, 0:QBS // 2], ops[:, 0:QBS // 2], Copy)
                        nc.vector.tensor_copy(osl[:, QBS // 2:], ops[:, QBS // 2:])
                        if tail and eb == 1:
                            nc.sync.dma_start(
                                out=out[st_idx * P:(st_idx + 1) * P, 0:2 * QBS],
                                in_=ost[:, 0:2 * QBS],
                            )
                    if tail:
                        nc.sync.dma_start(
                            out=out[st_idx * P:(st_idx + 1) * P, 2 * QBS:],
                            in_=ost[:, 2 * QBS:],
                        )
                    else:
                        nc.sync.dma_start(
                            out=out[st_idx * P:(st_idx + 1) * P, :], in_=ost
                        )

            OT_LAG = 3  # P.V matmul trails the score matmul so its sem wait
            # is already satisfied and LDWEIGHTS pipelines.
            blk = {}  # per-qb state carried from kt loop to den chain

            def att_ktloop(qb, op_qb=None):
                qsl = slice(qb * QBS, (qb + 1) * QBS)
                kmax = (qb + 1) * 4
                ot_pss, pts = [], {}
                # 4-lane bf16 partial sums of exp tiles (softmax denominator);
                # all adds run in the DVE 16-bit fast mode, chains stay short.
                accs = [[None] * 4 for _ in range(NHC)]

                def acc_pt(h, kt, pt, js):
                    # js = first valid column (fully-masked cols are skipped
                    # everywhere; zero-fill them once at lane init)
                    lane = kt % 4
                    if accs[h][lane] is None:
                        acc = work.tile(
                            [P, QBS], BF16, name=f"za{h}_{lane}",
                            tag=f"za{h}_{lane}",
                        )
                        if js:
                            nc.vector.memset(acc[:, 0:js], 0.0)
                        nc.vector.tensor_copy(acc[:, js:], pt[:, js:])
                        accs[h][lane] = acc
                    else:
                        acc = accs[h][lane]
                        nc.vector.tensor_add(acc[:, js:], acc[:, js:], pt[:, js:])

                for h in range(NHC):
                    ot_pss.append(ps_acc.tile([P, QBS], F32, name="ot_ps", tag="ps_ot"))

                kt_order = list(range(qb * 4, kmax)) + list(range(0, qb * 4))
                # spread the previous-previous block's out-proj groups through
                # this loop so their psum->sbuf copies never pile up
                op_at = {kmax // 2 - 1: 0, kmax - 3: 1} if op_qb is not None else {}

                def js_of(kt):
                    j = kt - qb * 4
                    return j * P if j > 0 else 0

                def pv_step(kt):
                    js = js_of(kt)
                    for h in range(NHC):
                        nc.tensor.matmul(
                            ot_pss[h][:, js:],
                            lhsT=v_sb[:, kt, h * HD:(h + 1) * HD],
                            rhs=pts[(h, kt)][:, js:],
                            start=(kt == kt_order[0]),
                            stop=(kt == kt_order[-1]),
                            skip_group_check=True,
                        )

                for ki, kt in enumerate(kt_order):
                    j = kt - qb * 4
                    js = js_of(kt)
                    for h in range(NHC):
                        sps = ps_score.tile([P, QBS], F32, tag="mm")
                        nc.tensor.matmul(
                            sps[:, js:],
                            lhsT=kt_sb[:, h, kt * P:(kt + 1) * P],
                            rhs=qt_sb[:, h, qb * QBS + js:(qb + 1) * QBS],
                            start=True,
                            stop=True,
                        )
                        pt = ptpool.tile([P, QBS], BF16, tag=f"pt{h}")
                        nc.scalar.activation(pt[:, js:], sps[:, js:], Exp, scale=SCALE)
                        if j >= 0:  # partial strip of the diagonal tile
                            nc.vector.tensor_mul(
                                pt[:, js:js + P], pt[:, js:js + P],
                                cm_sb[:, j, js:js + P],
                            )
                        pts[(h, kt)] = pt
                        acc_pt(h, kt, pt, js)
                    if ki >= OT_LAG:
                        pv_step(kt_order[ki - OT_LAG])
                    if ki in op_at:
                        out_proj(op_qb, s4s=(op_at[ki],))
                for ki2 in range(max(kmax - OT_LAG, 0), kmax):
                    pv_step(kt_order[ki2])
                blk[qb] = (ot_pss, accs)

            def att_den(qb):
                qsl = slice(qb * QBS, (qb + 1) * QBS)
                ot_pss, accs = blk.pop(qb)
                den_sbs = []
                for h in range(NHC):
                    # merge the 4 bf16 lanes on DVE, then one ones-matmul
                    lanes = [a for a in accs[h] if a is not None]
                    while len(lanes) > 1:
                        nxt = []
                        for i in range(0, len(lanes) - 1, 2):
                            nc.vector.tensor_add(lanes[i], lanes[i], lanes[i + 1])
                            nxt.append(lanes[i])
                        if len(lanes) % 2:
                            nxt.append(lanes[-1])
                        lanes = nxt
                    den_ps = ps_op.tile([P, QBS], F32, name="den_ps", tag="op")
                    nc.tensor.matmul(
                        den_ps[0:1, :], lhsT=ones_colb, rhs=lanes[0],
                        start=True, stop=True, skip_group_check=True,
                    )
                    # 1/x as exp(-ln(x)) on the scalar engine: the sanctioned
                    # DVE reciprocal runs 1 lane * 512 elems ~ 3.3us, far too
                    # slow for the tail critical path.
                    lnd = work.tile([1, QBS], F32, tag=f"ln{h}")
                    nc.scalar.activation(lnd, den_ps[0:1, :], Ln)
                    r_bf = work.tile([1, QBS], BF16, tag=f"rb{h}")
                    nc.scalar.activation(r_bf, lnd, Exp, scale=-1.0)
                    den_sbs.append(r_bf)
                for h in range(NHC):
                    bc_ps = ps_op.tile([P, QBS], F32, name="bc_ps", tag="op")
                    nc.tensor.matmul(
                        bc_ps, lhsT=ones_row, rhs=den_sbs[h], start=True, stop=True
                    )
                    bc_sb = work.tile([P, QBS], F32, tag=f"bc{h}")
                    nc.vector.tensor_copy(bc_sb, bc_ps)
                    nc.vector.tensor_mul(otb_sb[:, h, qsl], ot_pss[h], bc_sb)

            # ---- main schedule: den chains hide under later PE work ----
            proj_block(0, warm=True)
            att_ktloop(0)
            proj_block(1)
            att_den(0)
            att_ktloop(1)
            proj_block(2)
            att_den(1)
            att_ktloop(2, op_qb=0)
            out_proj(0, s4s=(2, 3))
            proj_block(3)
            att_den(2)
            att_ktloop(3, op_qb=1)
            # tail: scores are done, so the 4-slot score psum pool is free —
            # use it for the final out-projections to avoid 2-slot stalls
            out_proj(1, s4s=(2, 3), pool=ps_score)  # fills PE under den(3) DVE
            att_den(3)
            out_proj(2, pool=ps_score)
            out_proj(3, tail=True, pool=ps_score)
    if split_waits:
        _split_excess_waits(nc)
    return nc


_NC_CACHE = {}


def _get_nc():
    if "nc" not in _NC_CACHE:
        _NC_CACHE["nc"] = build_nc()
    return _NC_CACHE["nc"]


def _rotation_matrix_T():
    # rot(x)[2i] = -x[2i+1]; rot(x)[2i+1] = x[2i].  R[i,j] coefficient of x[j].
    R = np.zeros((HD, HD), np.float32)
    idx = np.arange(0, HD, 2)
    R[idx, idx + 1] = -1.0
    R[idx + 1, idx] = 1.0
    return np.ascontiguousarray(R.T)


def prepare_in_maps(hidden_states, sin, cos, Wq, Wk, Wv, Wo):
    hidden_states = np.asarray(hidden_states, dtype=np.float32)
    sin = np.asarray(sin, dtype=np.float32)
    cos = np.asarray(cos, dtype=np.float32)
    Wq = np.asarray(Wq, dtype=np.float32)
    Wk = np.asarray(Wk, dtype=np.float32)
    Wv = np.asarray(Wv, dtype=np.float32)
    Wo = np.asarray(Wo, dtype=np.float32)

    hT = hidden_states[0].T.astype(NPBF16)  # [D, S]
    hT4 = np.ascontiguousarray(
        hT.reshape(KD, P, NQB, QBS).transpose(1, 2, 0, 3)
    )  # [P, NQB, KD, QBS]
    ct = np.repeat(cos, 2, axis=1).T  # [P, S]
    st = np.repeat(sin, 2, axis=1).T
    rope = np.ascontiguousarray(
        np.stack(
            [ct.reshape(P, NQB, QBS), st.reshape(P, NQB, QBS)], axis=2
        )
    ).astype(NPBF16)  # [P, NQB, 2, QBS]
    rot = _rotation_matrix_T().astype(NPBF16)
    kk, jj, xx = np.meshgrid(
        np.arange(P), np.arange(4), np.arange(QBS), indexing="ij"
    )
    cm = (xx >= jj * P + kk).astype(NPBF16)  # [P, 4, QBS]

    in_maps = []
    for c in range(N_CORES):
        e0 = c * DC
        wq_p = np.ascontiguousarray(
            Wq[e0:e0 + DC, :].T.astype(NPBF16).reshape(KD, P, DC).transpose(1, 0, 2)
        )
        wk_p = np.ascontiguousarray(
            Wk[e0:e0 + DC, :].T.astype(NPBF16).reshape(KD, P, DC).transpose(1, 0, 2)
        )
        wv_p = np.ascontiguousarray(
            Wv[e0:e0 + DC, :].T.astype(NPBF16).reshape(KD, P, DC).transpose(1, 0, 2)
        )
        wo_p = np.ascontiguousarray(
            Wo[:, e0:e0 + DC].T.astype(NPBF16).reshape(NHC, P, D).transpose(1, 0, 2)
        )
        in_maps.append(
            {
                "hT": hT4,
                "wq": wq_p,
                "wk": wk_p,
                "wv": wv_p,
                "wo": wo_p,
                "rope": rope,
                "rot": rot,
                "cmask": cm,
            }
        )
    return in_maps


def kernel(hidden_states, attention_mask, sin, cos, Wq, Wk, Wv, Wo):
    in_maps = prepare_in_maps(hidden_states, sin, cos, Wq, Wk, Wv, Wo)
    nc = _get_nc()
    res = run_bass_kernel_spmd(nc, in_maps, list(range(N_CORES)))
    out = res.results[0]["out"].astype(np.float32)
    for c in range(1, N_CORES):
        out += res.results[c]["out"].astype(np.float32)
    return out[None]


# revision 21
# speedup vs baseline: 1.2265x; 1.0137x over previous
"""GPT-J joint attention (B=1, S=2048, D=2048, H=16, HD=128) on 8 Trainium2
NeuronCores, tensor-parallel over heads (2 heads per core).

Per-core program (all matmuls bf16 inputs, fp32 PSUM accumulation):
  - QT/KT = W[qk]_shard @ hidden^T        ([hd, s] layout, per head)
  - RoPE applied via a rotation-matrix matmul + elementwise combine
  - V = hidden @ Wv_shard^T               ([s, hd] layout)
  - scores^T tiles = KT_tile^T . QT_block ([k, q] layout) -> exp -> causal
    mask via a precomputed 0/1 mask multiply on DVE
  - O^T accumulated as V_tile^T . P^T; softmax denominator via ones-matmul
  - partial out = O^T{normalized}^T . Wo_shard^T, streamed to DRAM per row-block

v3 structure: host pre-lays-out all inputs so every DMA is contiguous; a
dummy-matmul warmup chain un-throttles the PE clock (HAM) while the first
weights stream in; Q/K projection chains for both heads interleave per-kd so
chunked hT DMA keeps up; attention for block qb runs right after its
projections; each block's softmax-denominator chain is emitted after later
independent PE work so the PE never head-of-line blocks on DVE; out-proj
row-groups are spread through the next attention loops to smooth the
psum->sbuf copy load.

Host side: shard/transpose/cast inputs, run SPMD on 8 cores, sum the 8
partial outputs (the tensor-parallel all-reduce equivalent).
"""
import sys

import numpy as np
import ml_dtypes

try:
    import concourse.bass as bass
except ImportError:  # pragma: no cover
    sys.path.insert(0, "/opt/trn_rl_repo")
    import concourse.bass as bass

import concourse.mybir as mybir
import concourse.tile as tile
from concourse.bass_utils import run_bass_kernel_spmd

BF16 = mybir.dt.bfloat16
F32 = mybir.dt.float32
NPBF16 = ml_dtypes.bfloat16

N_CORES = 8
S = 2048          # sequence length
D = 2048          # model dim
HD = 128          # head dim
NHC = 2           # heads per core
DC = NHC * HD     # shard width (256)
P = 128           # partitions
KD = D // P       # 16 contraction tiles over model dim
QBS = 512         # q-block size
NQB = S // QBS    # 4 q-blocks
NST = S // P      # 16 sequence tiles of 128
SCALE = 1.0 / float(np.sqrt(HD))
N_WARM = 60       # dummy matmuls covering the pre-DMA window (~55ns each cold)

# ---------------------------------------------------------------------------
# Walrus's CoreV3 drain encoding accepts a single sem wait; Tile's tail drain
# carries one wait per logical proc. Split it into one drain per proc.
# ---------------------------------------------------------------------------


def _install_drain_split():
    if getattr(tile.TileContext, "_drain_split_installed", False):
        return
    from concourse.vector_clock import ScopedClock, VectorClock

    def _drain_and_barrier(self, tick_clock, wait_clock):
        full = tick_clock.global_clock
        n = len(full)
        for i in range(n):
            if full[i] <= 0:
                continue
            vec = [full[j] if j == i else 0 for j in range(n)]
            drain_inst = self.nc.sync.drain()
            wait_clock.add_sem_waits(
                drain_inst.ins, ScopedClock({None: VectorClock(vec)})
            )
        self.nc.all_engine_barrier()
        assert self.sems is not None
        popped = self.nc._tile_sem_poison_stack.pop()
        assert popped is self._sem_poison
        self.nc.clear_and_free_semaphores(list(self.sems.allocated().values()))
        self.nc.all_engine_barrier()

    tile.TileContext._drain_and_barrier = _drain_and_barrier
    tile.TileContext._drain_split_installed = True


def _split_excess_waits(nc, limit=1):
    """This walrus build rejects instructions carrying more than one sem wait
    (CoreV3 setupSyncWait: 'Too many sync wait commands'). Spill excess waits
    onto same-engine NOPs inserted just before the instruction — the engine
    executes them in queue order, so blocking semantics are unchanged."""
    ctr = 0
    for fn in nc.m.functions:
        for blk in fn.blocks:
            new_list = []
            for inst in blk.instructions:
                si = inst.sync_info
                if si is not None and len(si.on_wait) > limit:
                    waits = list(si.on_wait)
                    excess, keep = waits[:-limit], waits[-limit:]
                    for w in excess:
                        ctr += 1
                        nop = mybir.InstNoOp(
                            name=f"I-wsplit-{ctr}", text_hint="wait_split"
                        )
                        nop.engine = inst.engine
                        nop.sync_info = mybir.SyncInfo(on_wait=[w], on_update=[])
                        new_list.append(nop)
                    inst.sync_info = mybir.SyncInfo(
                        on_wait=keep, on_update=si.on_update
                    )
                new_list.append(inst)
            if len(new_list) != len(blk.instructions):
                blk.instructions[:] = new_list
    return ctr


def build_nc(split_waits=True):
    _install_drain_split()
    nc = bass.Bass()

    # All inputs are host-pre-laid-out so each DMA below is a contiguous copy.
    hT = nc.dram_tensor("hT", [P, NQB, KD, QBS], BF16, kind="ExternalInput")
    wq = nc.dram_tensor("wq", [P, KD, DC], BF16, kind="ExternalInput")
    wk = nc.dram_tensor("wk", [P, KD, DC], BF16, kind="ExternalInput")
    wv = nc.dram_tensor("wv", [P, KD, DC], BF16, kind="ExternalInput")
    wo = nc.dram_tensor("wo", [P, NHC, D], BF16, kind="ExternalInput")
    # rope[p, qb, 0, :] = cos row p of q-block qb; rope[p, qb, 1, :] = sin
    rope = nc.dram_tensor("rope", [P, NQB, 2, QBS], BF16, kind="ExternalInput")
    rot = nc.dram_tensor("rot", [P, P], BF16, kind="ExternalInput")
    # cmask[k, j, x] = 1.0 if x >= j*128 + k else 0 (causal mask, diag tile j)
    cmask = nc.dram_tensor("cmask", [P, 4, QBS], BF16, kind="ExternalInput")
    out = nc.dram_tensor("out", [S, D], BF16, kind="ExternalOutput")

    Exp = mybir.ActivationFunctionType.Exp
    Copy = mybir.ActivationFunctionType.Copy
    Ln = mybir.ActivationFunctionType.Ln

    with tile.TileContext(nc) as tc:
        with (
            tc.tile_pool(name="const", bufs=1) as const,
            tc.tile_pool(name="acts", bufs=1) as acts,
            tc.tile_pool(name="work", bufs=2) as work,
            tc.tile_pool(name="ptpool", bufs=6) as ptpool,
            tc.tile_pool(name="outstage", bufs=3) as outstage,
            tc.tile_pool(name="ps_score", bufs=4, space="PSUM") as ps_score,
            tc.tile_pool(name="ps_op", bufs=2, space="PSUM") as ps_op,
            tc.tile_pool(name="ps_acc", bufs=2, space="PSUM") as ps_acc,
        ):
            # ---- PE warmup: HAM un-throttles after ~3.4us of sustained PE
            # activity; run a dummy matmul chain while the first DMAs land so
            # the real projection stream starts at full clock. ----
            wdum = const.tile([P, 64], BF16)
            nc.vector.memset(wdum, 0.25)
            wps = ps_acc.tile([P, QBS], F32, name="warm", tag="ps_ot")

            def dummy(n):
                for _ in range(n):
                    nc.tensor.matmul(
                        wps[0:64, 0:64], lhsT=wdum, rhs=wdum[:, 0:64],
                        start=True, stop=True, skip_group_check=True,
                    )

            dummy(N_WARM)

            # ---- constants / weights into SBUF; order = first-need order ----
            wq_sb = const.tile([P, KD, DC], BF16)
            wk_sb = const.tile([P, KD, DC], BF16)
            wv_sb = const.tile([P, KD, DC], BF16)
            hT_sb = const.tile([P, NQB, KD, QBS], BF16)
            rope_sb = const.tile([P, NQB, 2, QBS], BF16)
            rot_sb = const.tile([P, P], BF16)
            cm_sb = const.tile([P, 4, QBS], BF16)
            wo_sb = const.tile([P, NHC, D], BF16)
            nc.sync.dma_start(out=wq_sb[:, 0:8, :], in_=wq[:, 0:8, :])
            nc.sync.dma_start(out=hT_sb[:, 0, 0:4, :], in_=hT[:, 0, 0:4, :])
            nc.sync.dma_start(out=hT_sb[:, 0, 4:8, :], in_=hT[:, 0, 4:8, :])
            nc.sync.dma_start(out=wq_sb[:, 8:16, :], in_=wq[:, 8:16, :])
            for c4 in range(2, 4):
                nc.sync.dma_start(
                    out=hT_sb[:, 0, c4 * 4:(c4 + 1) * 4, :],
                    in_=hT[:, 0, c4 * 4:(c4 + 1) * 4, :],
                )
            nc.sync.dma_start(out=wk_sb, in_=wk[:, :, :])
            nc.sync.dma_start(out=rot_sb, in_=rot[:, :])
            nc.sync.dma_start(out=rope_sb[:, 0], in_=rope[:, 0])
            nc.sync.dma_start(out=wv_sb, in_=wv[:, :, :])
            nc.sync.dma_start(out=cm_sb, in_=cmask[:, :, :])
            for qb in range(1, NQB):
                nc.sync.dma_start(out=hT_sb[:, qb], in_=hT[:, qb])
                nc.sync.dma_start(out=rope_sb[:, qb], in_=rope[:, qb])
            nc.sync.dma_start(out=wo_sb, in_=wo[:, :, :])
            ones_colb = const.tile([P, 1], BF16)   # lhsT for k-partition sums
            nc.vector.memset(ones_colb, 1.0)
            ones_row = const.tile([1, P], BF16)    # lhsT for partition broadcast
            nc.vector.memset(ones_row, 1.0)

            # persistent activations
            qt_sb = acts.tile([P, NHC, S], BF16)   # [hd, h, s] rotary-applied Q^T
            kt_sb = acts.tile([P, NHC, S], BF16)
            v_sb = acts.tile([P, NST, DC], BF16)   # [s%128, s//128, head*hd]
            otb_sb = acts.tile([P, NHC, S], BF16)  # normalized O^T per head

            # ---- projections; rope epilogues (psum->sbuf copy, rotation
            # matmul, 3 DVE ops) are deferred and spread over the V chains ----
            pending = []  # (psum, dst_sb, h, qb)
            flush_ctr = [0]

            def flush_rope(keep=0):
                while len(pending) > keep:
                    ps, dst_sb, h, qb = pending.pop(0)
                    sl = slice(qb * QBS, (qb + 1) * QBS)
                    raw = work.tile([P, QBS], BF16, tag="raw")
                    # alternate the psum->sbuf copy between ACT and DVE so
                    # back-to-back flushes don't serialize on one engine
                    if flush_ctr[0] % 2 == 0:
                        nc.scalar.activation(raw, ps, Copy)
                    else:
                        nc.vector.tensor_copy(raw, ps)
                    flush_ctr[0] += 1
                    rps = ps_score.tile([P, QBS], F32, tag="mm")
                    nc.tensor.matmul(rps, lhsT=rot_sb, rhs=raw, start=True, stop=True)
                    t1 = work.tile([P, QBS], BF16, tag="t1")
                    t2 = work.tile([P, QBS], BF16, tag="t2")
                    nc.vector.tensor_mul(t1, raw, rope_sb[:, qb, 0])
                    nc.vector.tensor_mul(t2, rps, rope_sb[:, qb, 1])
                    nc.vector.tensor_add(dst_sb[:, h, sl], t1, t2)

            def qk_pair(w_sb, dst_sb, qb, warm=False):
                # both heads' 16-matmul chains interleaved per-kd so chunked
                # hT DMA delivery keeps up with PE consumption
                pss = [
                    ps_score.tile([P, QBS], F32, name=f"pp{h}", tag="mm")
                    for h in range(NHC)
                ]
                for kd in range(KD):
                    for h in range(NHC):
                        nc.tensor.matmul(
                            pss[h],
                            lhsT=w_sb[:, kd, h * HD:(h + 1) * HD],
                            rhs=hT_sb[:, qb, kd, :],
                            start=(kd == 0),
                            stop=(kd == KD - 1),
                        )
                    if warm and kd in (3, 7, 11):
                        dummy(12)
                for h in range(NHC):
                    pending.append((pss[h], dst_sb, h, qb))

            def proj_block(qb, den_prev=None, warm=False):
                qk_pair(wq_sb, qt_sb, qb, warm=warm)
                if warm:
                    dummy(10)
                if den_prev is not None:
                    # den-sum of the previous block here: its scalar Ln/Exp
                    # chain hides under the K-projection matmuls, so the bc
                    # matmuls (after the V chains) never wait on it
                    att_den_sum(den_prev)
                qk_pair(wk_sb, kt_sb, qb)
                for s4 in range(4):
                    # flush all rope epilogues by s4=2 so the attention loop's
                    # first (diagonal) score matmul never waits on kt_sb
                    flush_rope(keep=(3, 1, 0, 0)[s4])
                    st_idx = qb * 4 + s4
                    ps = ps_op.tile([P, QBS], F32, tag="op")
                    for kd in range(KD):
                        nc.tensor.matmul(
                            ps[:, 0:DC],
                            lhsT=hT_sb[:, qb, kd, s4 * P:(s4 + 1) * P],
                            rhs=wv_sb[:, kd, :],
                            start=(kd == 0),
                            stop=(kd == KD - 1),
                        )
                    nc.scalar.activation(v_sb[:, st_idx, :], ps[:, 0:DC], Copy)
                flush_rope()
                if den_prev is not None:
                    att_den_bc(den_prev)

            # ---- out-projection row-groups; psum->sbuf copies split in half
            # across ACT and DVE so the psum slot frees in ~350ns ----
            ost_open = {}

            def out_proj_step(qb, s4, eb):
                st_idx = qb * 4 + s4
                if eb == 0:
                    ost_open[(qb, s4)] = outstage.tile(
                        [P, D], BF16, name="ost", tag="ost"
                    )
                ost = ost_open[(qb, s4)]
                ops = ps_op.tile([P, QBS], F32, name="ops", tag="op")
                for h in range(NHC):
                    nc.tensor.matmul(
                        ops,
                        lhsT=otb_sb[:, h, st_idx * P:(st_idx + 1) * P],
                        rhs=wo_sb[:, h, eb * QBS:(eb + 1) * QBS],
                        start=(h == 0),
                        stop=(h == NHC - 1),
                    )
                osl = ost[:, eb * QBS:(eb + 1) * QBS]
                nc.scalar.activation(osl[:, 0:QBS // 2], ops[:, 0:QBS // 2], Copy)
                nc.vector.tensor_copy(osl[:, QBS // 2:], ops[:, QBS // 2:])
                if eb == NQB - 1:
                    nc.sync.dma_start(
                        out=out[st_idx * P:(st_idx + 1) * P, :], in_=ost
                    )
                    del ost_open[(qb, s4)]

            def out_proj(qb, s4s=range(4), tail=False, pool=None):
                pool = pool or ps_op
                ptag = "op" if pool is ps_op else "mm"
                for s4 in s4s:
                    st_idx = qb * 4 + s4
                    ost = outstage.tile([P, D], BF16, tag="ost")
                    for eb in range(NQB):
                        ops = pool.tile([P, QBS], F32, name="ops", tag=ptag)
                        for h in range(NHC):
                            nc.tensor.matmul(
                                ops,
                                lhsT=otb_sb[:, h, st_idx * P:(st_idx + 1) * P],
                                rhs=wo_sb[:, h, eb * QBS:(eb + 1) * QBS],
                                start=(h == 0),
                                stop=(h == NHC - 1),
                            )
                        osl = ost[:, eb * QBS:(eb + 1) * QBS]
                        nc.scalar.activation(osl[:, 0:QBS // 2], ops[:, 0:QBS // 2], Copy)
                        nc.vector.tensor_copy(osl[:, QBS // 2:], ops[:, QBS // 2:])
                        if tail and eb == 1:
                            nc.sync.dma_start(
                                out=out[st_idx * P:(st_idx + 1) * P, 0:2 * QBS],
                                in_=ost[:, 0:2 * QBS],
                            )
                    if tail:
                        nc.sync.dma_start(
                            out=out[st_idx * P:(st_idx + 1) * P, 2 * QBS:],
                            in_=ost[:, 2 * QBS:],
                        )
                    else:
                        nc.sync.dma_start(
                            out=out[st_idx * P:(st_idx + 1) * P, :], in_=ost
                        )

            OT_LAG = 3  # P.V matmul trails the score matmul so its sem wait
            # is already satisfied and LDWEIGHTS pipelines.
            blk = {}  # per-qb state carried from kt loop to den chain

            def att_ktloop(qb, op_qb=None):
                qsl = slice(qb * QBS, (qb + 1) * QBS)
                kmax = (qb + 1) * 4
                ot_pss, pts = [], {}
                # 4-lane bf16 partial sums of exp tiles (softmax denominator);
                # all adds run in the DVE 16-bit fast mode, chains stay short.
                accs = [[None] * 4 for _ in range(NHC)]

                def acc_pt(h, kt, pt, js):
                    # js = first valid column (fully-masked cols are skipped
                    # everywhere; zero-fill them once at lane init)
                    lane = kt % 4
                    if accs[h][lane] is None:
                        acc = work.tile(
                            [P, QBS], BF16, name=f"za{h}_{lane}",
                            tag=f"za{h}_{lane}",
                        )
                        if js:
                            nc.vector.memset(acc[:, 0:js], 0.0)
                        nc.vector.tensor_copy(acc[:, js:], pt[:, js:])
                        accs[h][lane] = acc
                    else:
                        acc = accs[h][lane]
                        nc.vector.tensor_add(acc[:, js:], acc[:, js:], pt[:, js:])

                for h in range(NHC):
                    ot_pss.append(ps_acc.tile([P, QBS], F32, name="ot_ps", tag="ps_ot"))

                kt_order = list(range(qb * 4, kmax)) + list(range(0, qb * 4))
                # spread the previous-previous block's out-proj work through
                # this loop one eb-pair per kt step, so neither the 2-slot
                # psum pool nor the copy engines ever back up
                op_at = {}
                if op_qb is not None:
                    for g, base in enumerate((kmax // 2 - 2, kmax - 4)):
                        for eb in range(NQB):
                            op_at[base + eb] = (g, eb)

                def js_of(kt):
                    j = kt - qb * 4
                    return j * P if j > 0 else 0

                def pv_step(kt):
                    js = js_of(kt)
                    for h in range(NHC):
                        nc.tensor.matmul(
                            ot_pss[h][:, js:],
                            lhsT=v_sb[:, kt, h * HD:(h + 1) * HD],
                            rhs=pts[(h, kt)][:, js:],
                            start=(kt == kt_order[0]),
                            stop=(kt == kt_order[-1]),
                            skip_group_check=True,
                        )

                for ki, kt in enumerate(kt_order):
                    j = kt - qb * 4
                    js = js_of(kt)
                    for h in range(NHC):
                        sps = ps_score.tile([P, QBS], F32, tag="mm")
                        nc.tensor.matmul(
                            sps[:, js:],
                            lhsT=kt_sb[:, h, kt * P:(kt + 1) * P],
                            rhs=qt_sb[:, h, qb * QBS + js:(qb + 1) * QBS],
                            start=True,
                            stop=True,
                        )
                        pt = ptpool.tile([P, QBS], BF16, tag=f"pt{h}")
                        nc.scalar.activation(pt[:, js:], sps[:, js:], Exp, scale=SCALE)
                        if j >= 0:  # partial strip of the diagonal tile
                            nc.vector.tensor_mul(
                                pt[:, js:js + P], pt[:, js:js + P],
                                cm_sb[:, j, js:js + P],
                            )
                        pts[(h, kt)] = pt
                        acc_pt(h, kt, pt, js)
                    if ki >= OT_LAG:
                        pv_step(kt_order[ki - OT_LAG])
                    if ki in op_at:
                        out_proj_step(op_qb, *op_at[ki])
                for ki2 in range(max(kmax - OT_LAG, 0), kmax):
                    pv_step(kt_order[ki2])
                blk[qb] = (ot_pss, accs)

            den_state = {}

            def att_den_sum(qb):
                ot_pss, accs = blk.pop(qb)
                den_sbs = []
                for h in range(NHC):
                    # merge the 4 bf16 lanes on DVE, then one ones-matmul
                    lanes = [a for a in accs[h] if a is not None]
                    while len(lanes) > 1:
                        nxt = []
                        for i in range(0, len(lanes) - 1, 2):
                            nc.vector.tensor_add(lanes[i], lanes[i], lanes[i + 1])
                            nxt.append(lanes[i])
                        if len(lanes) % 2:
                            nxt.append(lanes[-1])
                        lanes = nxt
                    den_ps = ps_op.tile([P, QBS], F32, name="den_ps", tag="op")
                    nc.tensor.matmul(
                        den_ps[0:1, :], lhsT=ones_colb, rhs=lanes[0],
                        start=True, stop=True, skip_group_check=True,
                    )
                    # 1/x as exp(-ln(x)) on the scalar engine: the sanctioned
                    # DVE reciprocal runs 1 lane * 512 elems ~ 3.3us, far too
                    # slow for the tail critical path.
                    lnd = work.tile([1, QBS], F32, tag=f"ln{h}")
                    nc.scalar.activation(lnd, den_ps[0:1, :], Ln)
                    r_bf = work.tile([1, QBS], BF16, tag=f"rb{h}")
                    nc.scalar.activation(r_bf, lnd, Exp, scale=-1.0)
                    den_sbs.append(r_bf)
                den_state[qb] = (ot_pss, den_sbs)

            def att_den_bc(qb):
                qsl = slice(qb * QBS, (qb + 1) * QBS)
                ot_pss, den_sbs = den_state.pop(qb)
                for h in range(NHC):
                    bc_ps = ps_op.tile([P, QBS], F32, name="bc_ps", tag="op")
                    nc.tensor.matmul(
                        bc_ps, lhsT=ones_row, rhs=den_sbs[h], start=True, stop=True
                    )
                    bc_sb = work.tile([P, QBS], F32, tag=f"bc{h}")
                    nc.vector.tensor_copy(bc_sb, bc_ps)
                    nc.vector.tensor_mul(otb_sb[:, h, qsl], ot_pss[h], bc_sb)

            # ---- main schedule: den chains hide under later PE work ----
            proj_block(0, warm=True)
            att_ktloop(0)
            proj_block(1, den_prev=0)
            att_ktloop(1)
            proj_block(2, den_prev=1)
            att_ktloop(2, op_qb=0)
            out_proj(0, s4s=(2, 3))
            proj_block(3, den_prev=2)
            att_ktloop(3, op_qb=1)
            # tail: scores are done, so the 4-slot score psum pool is free —
            # use it for the final out-projections to avoid 2-slot stalls
            out_proj(1, s4s=(2, 3), pool=ps_score)  # fills PE under den(3) DVE
            att_den_sum(3)
            out_proj(2, s4s=(0, 1), pool=ps_score)  # hides den(3) Ln/Exp chain
            att_den_bc(3)
            out_proj(2, s4s=(2, 3), pool=ps_score)
            out_proj(3, tail=True, pool=ps_score)
    if split_waits:
        _split_excess_waits(nc)
    return nc


_NC_CACHE = {}


def _get_nc():
    if "nc" not in _NC_CACHE:
        _NC_CACHE["nc"] = build_nc()
    return _NC_CACHE["nc"]


def _rotation_matrix_T():
    # rot(x)[2i] = -x[2i+1]; rot(x)[2i+1] = x[2i].  R[i,j] coefficient of x[j].
    R = np.zeros((HD, HD), np.float32)
    idx = np.arange(0, HD, 2)
    R[idx, idx + 1] = -1.0
    R[idx + 1, idx] = 1.0
    return np.ascontiguousarray(R.T)


def prepare_in_maps(hidden_states, sin, cos, Wq, Wk, Wv, Wo):
    hidden_states = np.asarray(hidden_states, dtype=np.float32)
    sin = np.asarray(sin, dtype=np.float32)
    cos = np.asarray(cos, dtype=np.float32)
    Wq = np.asarray(Wq, dtype=np.float32)
    Wk = np.asarray(Wk, dtype=np.float32)
    Wv = np.asarray(Wv, dtype=np.float32)
    Wo = np.asarray(Wo, dtype=np.float32)

    hT = hidden_states[0].T.astype(NPBF16)  # [D, S]
    hT4 = np.ascontiguousarray(
        hT.reshape(KD, P, NQB, QBS).transpose(1, 2, 0, 3)
    )  # [P, NQB, KD, QBS]
    ct = np.repeat(cos, 2, axis=1).T  # [P, S]
    st = np.repeat(sin, 2, axis=1).T
    rope = np.ascontiguousarray(
        np.stack(
            [ct.reshape(P, NQB, QBS), st.reshape(P, NQB, QBS)], axis=2
        )
    ).astype(NPBF16)  # [P, NQB, 2, QBS]
    rot = _rotation_matrix_T().astype(NPBF16)
    kk, jj, xx = np.meshgrid(
        np.arange(P), np.arange(4), np.arange(QBS), indexing="ij"
    )
    cm = (xx >= jj * P + kk).astype(NPBF16)  # [P, 4, QBS]

    in_maps = []
    for c in range(N_CORES):
        e0 = c * DC
        wq_p = np.ascontiguousarray(
            Wq[e0:e0 + DC, :].T.astype(NPBF16).reshape(KD, P, DC).transpose(1, 0, 2)
        )
        wk_p = np.ascontiguousarray(
            Wk[e0:e0 + DC, :].T.astype(NPBF16).reshape(KD, P, DC).transpose(1, 0, 2)
        )
        wv_p = np.ascontiguousarray(
            Wv[e0:e0 + DC, :].T.astype(NPBF16).reshape(KD, P, DC).transpose(1, 0, 2)
        )
        wo_p = np.ascontiguousarray(
            Wo[:, e0:e0 + DC].T.astype(NPBF16).reshape(NHC, P, D).transpose(1, 0, 2)
        )
        in_maps.append(
            {
                "hT": hT4,
                "wq": wq_p,
                "wk": wk_p,
                "wv": wv_p,
                "wo": wo_p,
                "rope": rope,
                "rot": rot,
                "cmask": cm,
            }
        )
    return in_maps


def kernel(hidden_states, attention_mask, sin, cos, Wq, Wk, Wv, Wo):
    in_maps = prepare_in_maps(hidden_states, sin, cos, Wq, Wk, Wv, Wo)
    nc = _get_nc()
    res = run_bass_kernel_spmd(nc, in_maps, list(range(N_CORES)))
    out = res.results[0]["out"].astype(np.float32)
    for c in range(1, N_CORES):
        out += res.results[c]["out"].astype(np.float32)
    return out[None]
